# revision 28
# baseline (speedup 1.0000x reference)
"""Trainium2 Bass kernel for nn_ANEPrefillLayer (dense transformer prefill layer).

Tensor-parallel over 8 NeuronCores:
  - q heads: 2 per core; kv head: 1 per core
  - o-proj column-sharded -> AllReduce of attention-output partials
  - MLP intermediate sharded (768 per core) -> ReduceScatter of (mlp + hm/8),
    each core emitting a 256-row feature shard of the final hidden state.
Matmul operands are cast to bf16 on chip (weights cast on host), fp32
accumulation in PSUM.  Activations kept feature-major (features on
partitions); q/k/v projections are produced token-major directly by using the
activation tiles as the stationary operand.
"""
import numpy as np

HID, H, HKV, D, INT, S, T = 2048, 16, 8, 128, 6144, 4096, 512
EPS = 1e-6
SCALE = 1.0 / D**0.5
NC = 8
KT = HID // 128          # 16 k-tiles over hidden
TT = T // 128            # 4 token tiles
QH = H // NC             # 2 q heads per core
MI = INT // NC           # 768 intermediate per core
NG = 4                   # gate/up m-groups (3 m-tiles each)
DG = 8                   # down-proj m-groups (2 m-tiles each)

_CACHE = {}


def _patch_walrus_flags():
    # pair LDWEIGHTS with MATMULs (the default disables the optimization)
    return


def _build():
    import concourse.bass as bass
    import concourse.bass_isa as bass_isa
    import concourse.mybir as mybir
    import concourse.tile as tile
    from concourse.masks import make_identity
    from concourse.bass import ds, ts

    f32 = mybir.dt.float32
    bf16 = mybir.dt.bfloat16
    AF = mybir.ActivationFunctionType
    ALU = mybir.AluOpType

    from concourse import bacc
    nc = bacc.Bacc()

    # ---- DRAM parameters (per-core shards; same graph on all cores) ----
    p_x = nc.declare_dram_parameter("x", [HID, T], f32, isOutput=False)
    p_wqkvt = nc.declare_dram_parameter("wqkvt", [HID, 512], bf16, isOutput=False)
    p_wot = nc.declare_dram_parameter("wot", [256, HID], bf16, isOutput=False)
    # gate/up: column groups [g0 g1 g2 | u0 u1 u2 | g3 g4 g5 | u3 u4 u5]
    p_wgut = nc.declare_dram_parameter("wgut", [NG, HID, 384], bf16, isOutput=False)
    p_wdt = nc.declare_dram_parameter("wdt", [DG, MI, 256], bf16, isOutput=False)
    p_ln1 = nc.declare_dram_parameter("ln1w", [HID, 1], f32, isOutput=False)
    p_ln2 = nc.declare_dram_parameter("ln2w", [HID, 1], f32, isOutput=False)
    p_cosq = nc.declare_dram_parameter("cosq", [T, D], f32, isOutput=False)
    p_sinq = nc.declare_dram_parameter("sinq", [T, D], f32, isOutput=False)
    p_cosk = nc.declare_dram_parameter("cosk", [T, D], f32, isOutput=False)
    p_sink = nc.declare_dram_parameter("sink", [T, D], f32, isOutput=False)
    p_mask = nc.declare_dram_parameter("mask", [T, T], f32, isOutput=False)

    p_outh = nc.declare_dram_parameter("out_hid", [HID // NC, T], bf16, isOutput=True)
    p_outk = nc.declare_dram_parameter("out_k", [T, D], f32, isOutput=True)
    p_outv = nc.declare_dram_parameter("out_v", [T, D], f32, isOutput=True)

    groups = [list(range(NC))]

    with tile.TileContext(nc) as tc:
        with (
            tc.tile_pool(name="dram", bufs=1, space="DRAM") as dram,
            tc.tile_pool(name="const", bufs=1) as const,
            tc.tile_pool(name="xpool", bufs=1) as xpool,
            tc.tile_pool(name="wstream", bufs=8) as wstream,
            tc.tile_pool(name="scratch", bufs=2) as scratch,
            tc.tile_pool(name="small", bufs=8) as small,
            tc.tile_pool(name="attn", bufs=1) as attn,
            tc.tile_pool(name="psacc", bufs=2, space="PSUM") as psacc,
            tc.tile_pool(name="psgrp", bufs=3, space="PSUM") as psgrp,
            tc.tile_pool(name="pstp", bufs=2, space="PSUM") as pstp,
        ):
            # collective bounce buffers (tracked DRAM tiles)
            cc1_sizes = [1024, 1024]
            cc1_ins = [dram.tile([cc1_sizes[i], T], bf16, name=f"cc1_in{i}")
                       for i in range(2)]
            cc1_outs = [dram.tile([cc1_sizes[i], T], bf16, addr_space="Shared",
                                  name=f"cc1_out{i}") for i in range(2)]
            cc2_sizes = [1024, 512, 512]
            cc2_ins = [dram.tile([cc2_sizes[i], T], bf16, name=f"cc2_in{i}")
                       for i in range(3)]
            cc2_out = dram.tile([HID // NC, T], bf16)
            rinv_dram = dram.tile([T, 1], f32)

            x_prime = xpool.tile([128, KT, T], f32, tag="xres", name="x_sb")
            nc.sync.dma_start(out=x_prime[:, 0], in_=p_x[ts(0, 128), :])
            wqkv_sb = const.tile([128, KT, 512], bf16)
            nc.sync.dma_start(out=wqkv_sb[:, 0], in_=p_wqkvt[ts(0, 128), :])

            ident = const.tile([128, 128], bf16)
            make_identity(nc, ident[:])
            eps_col = const.tile([128, 1], f32)
            nc.vector.memset(eps_col[:], EPS)

            # per-feature norm weights: (2048,1) -> (128, 16) column tiles
            ln1_sb = const.tile([128, KT], f32)
            nc.sync.dma_start(out=ln1_sb[:], in_=p_ln1[:].rearrange("(k p) o -> p (k o)", p=128))
            ln2_sb = const.tile([128, KT], f32)
            nc.sync.dma_start(out=ln2_sb[:], in_=p_ln2[:].rearrange("(k p) o -> p (k o)", p=128))

            # ---- phase A: stream x, LN1 stats, xw = x*ln1_w (bf16) ----
            xw_sb = xpool.tile([128, KT, T], bf16, tag="xw")
            x_sb = x_prime
            ss_acc = attn.tile([128, T], f32, tag="ssacc")
            for k in range(KT):
                if k > 0:
                    nc.sync.dma_start(out=x_sb[:, k], in_=p_x[ts(k, 128), :])
                x_t = x_sb[:, k]
                xsq = scratch.tile([128, T], bf16, tag="xsq")
                nc.scalar.activation(xsq[:], x_t, AF.Square)
                if k == 0:
                    nc.vector.tensor_copy(ss_acc[:], xsq[:])
                else:
                    nc.vector.tensor_tensor(ss_acc[:], ss_acc[:], xsq[:], op=ALU.add)
                nc.vector.tensor_tensor(xw_sb[:, k], x_t, ln1_sb[:, ds(k, 1)].to_broadcast((128, T)), op=ALU.mult)

            # rinv broadcast (128, T): 1/sqrt(sum/HID + eps) on every partition
            rinv_bc = attn.tile([128, T], f32, tag="rbc")
            nc.gpsimd.partition_all_reduce(rinv_bc[:], ss_acc[:], channels=128,
                                           reduce_op=bass_isa.ReduceOp.add)
            nc.scalar.activation(rinv_bc[:], rinv_bc[:], AF.Sqrt,
                                 bias=eps_col[:], scale=1.0 / HID)
            nc.vector.reciprocal(rinv_bc[:], rinv_bc[:])
            # token-major (128, TT) copy via DRAM bounce (for the v output)
            nc.sync.dma_start(out=rinv_dram[:], in_=rinv_bc[0:1, :])
            rinv_tm = small.tile([128, TT], f32, tag="rtm")
            nc.sync.dma_start(out=rinv_tm[:],
                              in_=rinv_dram[:].rearrange("(t p) o -> p (t o)", p=128))

            # ---- phase B: QKV (token-major) + head RMS + RoPE ----
            for k in range(1, KT):
                nc.sync.dma_start(out=wqkv_sb[:, k], in_=p_wqkvt[ts(k, 128), :])

            # RoPE tables + attention mask, token-tiled
            cosq_sb = const.tile([128, TT, D], f32)
            sinq_sb = const.tile([128, TT, D], f32)
            cosk_sb = const.tile([128, TT, D], f32)
            sink_sb = const.tile([128, TT, D], f32)
            mask_sb = const.tile([128, TT, T], f32)
            for t in range(TT):
                nc.sync.dma_start(out=cosq_sb[:, t], in_=p_cosq[ts(t, 128), :])
                nc.sync.dma_start(out=sinq_sb[:, t], in_=p_sinq[ts(t, 128), :])
                nc.sync.dma_start(out=cosk_sb[:, t], in_=p_cosk[ts(t, 128), :])
                nc.sync.dma_start(out=sink_sb[:, t], in_=p_sink[ts(t, 128), :])
                nc.sync.dma_start(out=mask_sb[:, t], in_=p_mask[ts(t, 128), :])


            q_fm = attn.tile([128, QH, T], bf16)   # q feature-major per head
            rq_all = {}
            k_fm = attn.tile([128, T], bf16)
            v_tm = attn.tile([128, TT, D], bf16)   # v token-major (unscaled)

            def rms_rope(dst_bf, src_ap, cos_t, sin_t, tag, dst_f32=None,
                         defer_scale=False):
                # src_ap: (128 tokens, 128 dims) PSUM f32
                sq = scratch.tile([128, D], bf16, tag="rsq")
                ssq = small.tile([128, 1], f32, tag=tag + "ss")
                nc.scalar.activation(sq[:], src_ap, AF.Square, accum_out=ssq[:])
                rr = small.tile([128, 1], f32, tag=tag + "rr", name=f"rr_{tag}")
                nc.scalar.activation(rr[:], ssq[:], AF.Sqrt, bias=eps_col[:], scale=1.0 / D)
                nc.vector.reciprocal(rr[:], rr[:])
                t1 = scratch.tile([128, D], f32, tag="rt1")
                t2 = scratch.tile([128, D], f32, tag="rt2")
                if defer_scale:
                    # rope WITHOUT the rms scale (applied later via exp scale);
                    # runs concurrently with the ssq/sqrt/recip chain
                    nc.vector.tensor_tensor(t1[:], src_ap, cos_t, op=ALU.mult)
                    nc.vector.tensor_tensor(t2[:, 0:64], src_ap[:, 64:128],
                                            sin_t[:, 0:64], op=ALU.mult)
                    nc.vector.tensor_tensor(t2[:, 64:128], src_ap[:, 0:64],
                                            sin_t[:, 64:128], op=ALU.mult)
                else:
                    nc.vector.scalar_tensor_tensor(
                        t1[:], src_ap, rr[:], cos_t, op0=ALU.mult, op1=ALU.mult)
                    nc.vector.scalar_tensor_tensor(
                        t2[:, 0:64], src_ap[:, 64:128], rr[:], sin_t[:, 0:64],
                        op0=ALU.mult, op1=ALU.mult)
                    nc.vector.scalar_tensor_tensor(
                        t2[:, 64:128], src_ap[:, 0:64], rr[:], sin_t[:, 64:128],
                        op0=ALU.mult, op1=ALU.mult)
                if dst_f32 is not None:
                    nc.vector.tensor_tensor(dst_f32, t1[:], t2[:], op=ALU.add)
                    nc.vector.tensor_copy(dst_bf, dst_f32)
                else:
                    nc.vector.tensor_tensor(dst_bf, t1[:], t2[:], op=ALU.add)
                return rr

            for t in range(TT):
                qkv_ps = psgrp.tile([128, 512], f32, tag="grp")
                for k in range(KT):
                    nc.tensor.matmul(qkv_ps[:], xw_sb[:, k, ts(t, 128)], wqkv_sb[:, k],
                                     start=(k == 0), stop=(k == KT - 1))
                q_ps = qkv_ps
                kv_ps = qkv_ps[:, 256:512]

                # q heads: RoPE (rms scale deferred to the exp) -> feature-major
                for h in range(QH):
                    qr = scratch.tile([128, D], bf16, tag="qr")
                    rq = rms_rope(qr[:], qkv_ps[:, ds(h * 128, 128)],
                                  cosq_sb[:, t], sinq_sb[:, t], f"q{h}",
                                  defer_scale=True)
                    rq_all[(t, h)] = rq
                    qt_ps = pstp.tile([128, 128], bf16, tag="tps")
                    nc.tensor.transpose(qt_ps[:], qr[:], ident[:])
                    nc.vector.tensor_copy(q_fm[:, h, ts(t, 128)], qt_ps[:])

                # k head: RMS + RoPE -> out_k (f32) + feature-major (bf16)
                krb = scratch.tile([128, D], bf16, tag="krb")
                krf = scratch.tile([128, D], f32, tag="krf")
                rms_rope(krb[:], qkv_ps[:, 256:384], cosk_sb[:, t], sink_sb[:, t],
                         "k", dst_f32=krf[:])
                nc.sync.dma_start(out=p_outk[ts(t, 128), :], in_=krf[:])
                kt_ps = pstp.tile([128, 128], bf16, tag="tps")
                nc.tensor.transpose(kt_ps[:], krb[:], ident[:])
                nc.vector.tensor_copy(k_fm[:, ts(t, 128)], kt_ps[:])

                # v: unscaled bf16 for attention; ln1-scaled f32 for output
                nc.vector.tensor_copy(v_tm[:, t], qkv_ps[:, 384:512])
                vout = scratch.tile([128, D], f32, tag="vout")
                nc.vector.tensor_tensor(vout[:], qkv_ps[:, 384:512],
                                        rinv_tm[:, ds(t, 1)].to_broadcast((128, D)), op=ALU.mult)
                nc.sync.dma_start(out=p_outv[ts(t, 128), :], in_=vout[:])

            # ---- phase D: attention, heads interleaved, causal-trimmed ----
            # (masked scores are exactly exp(-1e4)=0 in f32, so computing only
            #  the visible lower-triangular tile strip is exact)
            o_fm = attn.tile([128, QH, T], bf16)
            pT_sbs = [attn.tile([128, TT, T], bf16, name=f"pT{h}", tag=f"pT{h}")
                      for h in range(QH)]
            for h in range(QH):
                nc.vector.memset(pT_sbs[h][:], 0.0)
            for t in range(TT):
                W = (t + 1) * 128
                for h in range(QH):
                    s_ps = psacc.tile([128, T], f32, tag="acc")
                    nc.tensor.matmul(s_ps[:, 0:W], q_fm[:, h, ts(t, 128)],
                                     k_fm[:, 0:W], start=True, stop=True)
                    s_sb = scratch.tile([128, T], f32, tag="ssb")
                    nc.vector.scalar_tensor_tensor(
                        s_sb[:, 0:W], s_ps[:, 0:W], SCALE, mask_sb[:, t, 0:W],
                        op0=ALU.mult, op1=ALU.add)
                    p_sb = scratch.tile([128, T], f32, tag="psb")
                    rowsum = small.tile([128, 1], f32, tag="rsum")
                    nc.scalar.activation(p_sb[:, 0:W], s_sb[:, 0:W], AF.Exp,
                                         scale=rq_all[(t, h)][:],
                                         accum_out=rowsum[:])
                    nc.vector.reciprocal(rowsum[:], rowsum[:])
                    pn = scratch.tile([128, T], bf16, tag="pn")
                    nc.vector.scalar_tensor_tensor(
                        pn[:, 0:W], p_sb[:, 0:W], rowsum[:], rinv_bc[:, 0:W],
                        op0=ALU.mult, op1=ALU.mult)
                    for st in range(t + 1):
                        pt_ps = pstp.tile([128, 128], bf16, tag="tps")
                        nc.tensor.transpose(pt_ps[:], pn[:, ts(st, 128)], ident[:])
                        nc.vector.tensor_copy(pT_sbs[h][:, st, ts(t, 128)], pt_ps[:])
            for h in range(QH):
                o_ps = psacc.tile([128, T], f32, tag="acc")
                for st in range(TT):
                    nc.tensor.matmul(o_ps[:], v_tm[:, st], pT_sbs[h][:, st],
                                     start=(st == 0), stop=(st == TT - 1))
                nc.vector.tensor_copy(o_fm[:, h], o_ps[:])

            # ---- phase E: o-proj partials -> AllReduce ----
            wo_sb = const.tile([128, 2, HID], bf16)
            for k in range(2):
                nc.sync.dma_start(out=wo_sb[:, k], in_=p_wot[ts(k, 128), :])
            cc1_mranges = [(0, 8), (8, 16)]
            for q, (m0, m1) in enumerate(cc1_mranges):
                for m in range(m0, m1):
                    ao_ps = psacc.tile([128, T], f32, tag="acc")
                    for k in range(2):
                        nc.tensor.matmul(ao_ps[:], wo_sb[:, k, ts(m, 128)], o_fm[:, k],
                                         start=(k == 0), stop=(k == 1))
                    ao_sb = scratch.tile([128, T], bf16, tag="aosb")
                    nc.any.tensor_copy(ao_sb[:], ao_ps[:])
                    nc.sync.dma_start(out=cc1_ins[q][ts(m - m0, 128), :], in_=ao_sb[:])
                nc.gpsimd.collective_compute(
                    "AllReduce", ALU.add,
                    ins=[cc1_ins[q][:]],
                    outs=[cc1_outs[q][:]],
                    replica_groups=groups)

            # ---- phase F: residual + LN2 + MLP + ReduceScatter ----
            hm_sb = xpool.tile([128, KT, T], f32, tag="hm")
            hw2_sb = xpool.tile([128, KT, T], bf16, tag="xw")   # reuse xw slot
            ss2_acc = attn.tile([128, T], f32, tag="ss2acc")
            for k in range(KT):
                x2 = x_sb[:, k]
                ao = scratch.tile([128, T], bf16, tag="aoin")
                nc.gpsimd.dma_start(out=ao[:], in_=cc1_outs[k // 8][ts(k % 8, 128), :])
                nc.vector.tensor_tensor(hm_sb[:, k], x2, ao[:], op=ALU.add)
                xsq2 = scratch.tile([128, T], bf16, tag="xsq")
                nc.scalar.activation(xsq2[:], hm_sb[:, k], AF.Square)
                if k == 0:
                    nc.vector.tensor_copy(ss2_acc[:], xsq2[:])
                else:
                    nc.vector.tensor_tensor(ss2_acc[:], ss2_acc[:], xsq2[:], op=ALU.add)
                nc.vector.tensor_tensor(hw2_sb[:, k], hm_sb[:, k],
                                        ln2_sb[:, ds(k, 1)].to_broadcast((128, T)),
                                        op=ALU.mult)

            rinv2_bc = attn.tile([128, T], f32, tag="r2bc")
            nc.gpsimd.partition_all_reduce(rinv2_bc[:], ss2_acc[:], channels=128,
                                           reduce_op=bass_isa.ReduceOp.add)
            nc.scalar.activation(rinv2_bc[:], rinv2_bc[:], AF.Sqrt,
                                 bias=eps_col[:], scale=1.0 / HID)
            nc.vector.reciprocal(rinv2_bc[:], rinv2_bc[:])

            # gate/up: 4 groups of 3 m-tiles (g g g | u u u | g g g | u u u)
            act_sb = attn.tile([128, MI // 128, T], bf16, tag="act")
            gate_tiles = {}
            for g in range(NG):
                gu_ps = [psgrp.tile([128, T], f32, tag="grp", name=f"gu_ps{g}_{i}") for i in range(3)]
                for k in range(KT):
                    wtile = wstream.tile([128, 384], bf16, tag="wgu")
                    nc.sync.dma_start(out=wtile[:], in_=p_wgut[g, ts(k, 128), :])
                    for mi in range(3):
                        nc.tensor.matmul(gu_ps[mi][:], wtile[:, ds(mi * 128, 128)],
                                         hw2_sb[:, k],
                                         start=(k == 0), stop=(k == KT - 1))
                is_gate = (g % 2 == 0)
                base = (g // 2) * 3
                for mi in range(3):
                    pre = scratch.tile([128, T], f32, tag="gupre")
                    nc.vector.tensor_tensor(pre[:], gu_ps[mi][:], rinv2_bc[:],
                                            op=ALU.mult)
                    if is_gate:
                        gt = attn.tile([128, T], f32, tag=f"gate{mi}")
                        nc.scalar.activation(gt[:], pre[:], AF.Silu)
                        gate_tiles[base + mi] = gt
                    else:
                        nc.vector.tensor_tensor(act_sb[:, base + mi],
                                                gate_tiles[base + mi][:], pre[:],
                                                op=ALU.mult)

            # down-proj: 8 groups of 2 m-tiles + residual/8 -> cc2_in
            # (RS chunk emitted after each half's groups)
            for g in range(DG):
                d_ps = [psgrp.tile([128, T], f32, tag="grp", name=f"d_ps{g}_{i}") for i in range(2)]
                for k in range(MI // 128):
                    wtile = wstream.tile([128, 256], bf16, tag="wd")
                    nc.sync.dma_start(out=wtile[:], in_=p_wdt[g, ts(k, 128), :])
                    for mi in range(2):
                        nc.tensor.matmul(d_ps[mi][:], wtile[:, ds(mi * 128, 128)],
                                         act_sb[:, k],
                                         start=(k == 0), stop=(k == MI // 128 - 1))
                for mi in range(2):
                    m = g * 2 + mi
                    fin = scratch.tile([128, T], bf16, tag="fin")
                    nc.vector.scalar_tensor_tensor(
                        fin[:], hm_sb[:, m], 1.0 / NC, d_ps[mi][:],
                        op0=ALU.mult, op1=ALU.add)
                    mq = 0 if m < 8 else (1 if m < 12 else 2)
                    nc.gpsimd.dma_start(out=cc2_ins[mq][ts(m - (0, 8, 12)[mq], 128), :], in_=fin[:])
                if g in (3, 5, 7):
                    q = (3, 5, 7).index(g)
                    out_off, out_n = ((0, 128), (128, 64), (192, 64))[q]
                    nc.gpsimd.collective_compute(
                        "ReduceScatter", ALU.add,
                        ins=[cc2_ins[q][:]],
                        outs=[cc2_out[out_off:out_off + out_n, :]],
                        replica_groups=groups)

            nc.sync.dma_start(out=p_outh[:], in_=cc2_out[:])

    nc.compile()
    return nc


def _get_nc():
    if "nc" not in _CACHE:
        _patch_walrus_flags()
        _CACHE["nc"] = _build()
    return _CACHE["nc"]


def _shard_inputs(hidden_conv, cos, sin, attn_mask, wq, wk, wv, wo,
                  ln1_w, ln2_w, qn_w, kn_w, w_gate_up, w_down):
    import ml_dtypes
    f = np.float32
    bf = ml_dtypes.bfloat16
    x_fm = np.ascontiguousarray(np.asarray(hidden_conv, f)[0, :, 0, :])   # (2048, 512)
    cos2 = np.asarray(cos, f)[0]
    sin2 = np.asarray(sin, f)[0]
    mask = np.ascontiguousarray(np.asarray(attn_mask, f)[0, 0, :, :T])    # (512, 512)
    qn = np.asarray(qn_w, f); kn = np.asarray(kn_w, f)
    ln1 = np.ascontiguousarray(np.asarray(ln1_w, f).reshape(HID, 1))
    ln2 = np.ascontiguousarray(np.asarray(ln2_w, f).reshape(HID, 1))
    wq = np.asarray(wq, f); wk = np.asarray(wk, f); wv = np.asarray(wv, f)
    wo = np.asarray(wo, f); wgu = np.asarray(w_gate_up, f); wd = np.asarray(w_down, f)

    def rope_tables(w):
        cosw = cos2 * w[None, :]
        sinw = np.concatenate(
            [-sin2[:, 0:64] * w[None, 64:128], sin2[:, 64:128] * w[None, 0:64]], axis=1)
        return np.ascontiguousarray(cosw), np.ascontiguousarray(sinw)

    cosq, sinq = rope_tables(qn)
    cosk, sink = rope_tables(kn)

    in_maps = []
    for c in range(NC):
        wqkvt = np.ascontiguousarray(
            np.concatenate([wq[c * 256:(c + 1) * 256], wk[c * 128:(c + 1) * 128],
                            wv[c * 128:(c + 1) * 128]], 0).T.astype(bf))
        wot = np.ascontiguousarray(wo[:, c * 256:(c + 1) * 256].T.astype(bf))
        # (2048, 1536) -> NG groups of 384 cols: [g0 g1 g2 | u0 u1 u2 | g3.. ]
        wg = wgu[c * MI:(c + 1) * MI].T            # (2048, 768) gate
        wu = wgu[INT + c * MI:INT + (c + 1) * MI].T
        wgut = np.stack([
            wg[:, 0:384], wu[:, 0:384], wg[:, 384:768], wu[:, 384:768]], 0)
        wgut = np.ascontiguousarray(wgut.astype(bf))           # (4, 2048, 384)
        wdt = wd[:, c * MI:(c + 1) * MI].T         # (768, 2048)
        wdt = np.ascontiguousarray(
            wdt.reshape(MI, DG, 256).transpose(1, 0, 2).astype(bf))  # (8, 768, 256)
        in_maps.append(dict(
            x=x_fm, wqkvt=wqkvt, wot=wot, wgut=wgut, wdt=wdt,
            ln1w=ln1, ln2w=ln2, cosq=cosq, sinq=sinq, cosk=cosk, sink=sink,
            mask=mask))
    return in_maps


def kernel(hidden_conv, cos, sin, update_mask, attn_mask, k_cache, v_cache,
           wq, wk, wv, wo, ln1_w, ln2_w, qn_w, kn_w, w_gate_up, w_down):
    import os
    from concourse.bass_utils import run_bass_kernel_spmd

    in_maps = _shard_inputs(hidden_conv, cos, sin, attn_mask, wq, wk, wv, wo,
                            ln1_w, ln2_w, qn_w, kn_w, w_gate_up, w_down)
    nc = _get_nc()
    trace = bool(os.environ.get("KERNEL_TRACE"))
    res = run_bass_kernel_spmd(nc, in_maps, core_ids=list(range(NC)), trace=trace)
    _CACHE["last_res"] = res
    results = res.results

    hid = np.empty((HID, T), np.float32)
    for c in range(NC):
        oh = np.asarray(results[c]["out_hid"], np.float32)
        hid[c * 128:(c + 1) * 128] = oh[0:128]
        hid[1024 + c * 64:1024 + (c + 1) * 64] = oh[128:192]
        hid[1536 + c * 64:1536 + (c + 1) * 64] = oh[192:256]
    hid = np.ascontiguousarray(hid.reshape(1, HID, 1, T))
    kf = np.zeros((1, HKV, S, D), np.float32)
    vf = np.zeros((1, HKV, S, D), np.float32)
    for c in range(NC):
        kf[0, c, :T] = results[c]["out_k"]
        vf[0, c, :T] = results[c]["out_v"]
    return hid, kf, vf


# revision 30
# speedup vs baseline: 1.0297x; 1.0297x over previous
"""Trainium2 Bass kernel for nn_ANEPrefillLayer (dense transformer prefill layer).

Tensor-parallel over 8 NeuronCores:
  - q heads: 2 per core; kv head: 1 per core
  - o-proj column-sharded -> AllReduce of attention-output partials
  - MLP intermediate sharded (768 per core) -> ReduceScatter of (mlp + hm/8),
    each core emitting a 256-row feature shard of the final hidden state.
Matmul operands are cast to bf16 on chip (weights cast on host), fp32
accumulation in PSUM.  Activations kept feature-major (features on
partitions); q/k/v projections are produced token-major directly by using the
activation tiles as the stationary operand.
"""
import numpy as np

HID, H, HKV, D, INT, S, T = 2048, 16, 8, 128, 6144, 4096, 512
EPS = 1e-6
SCALE = 1.0 / D**0.5
NC = 8
KT = HID // 128          # 16 k-tiles over hidden
TT = T // 128            # 4 token tiles
QH = H // NC             # 2 q heads per core
MI = INT // NC           # 768 intermediate per core
NG = 4                   # gate/up m-groups (3 m-tiles each)
DG = 8                   # down-proj m-groups (2 m-tiles each)

_CACHE = {}


def _patch_walrus_flags():
    # pair LDWEIGHTS with MATMULs (the default disables the optimization)
    return


def _build():
    import concourse.bass as bass
    import concourse.bass_isa as bass_isa
    import concourse.mybir as mybir
    import concourse.tile as tile
    from concourse.masks import make_identity
    from concourse.bass import ds, ts

    f32 = mybir.dt.float32
    bf16 = mybir.dt.bfloat16
    AF = mybir.ActivationFunctionType
    ALU = mybir.AluOpType

    from concourse import bacc
    nc = bacc.Bacc()

    # ---- DRAM parameters (per-core shards; same graph on all cores) ----
    p_x = nc.declare_dram_parameter("x", [HID, T], f32, isOutput=False)
    p_wqkvt = nc.declare_dram_parameter("wqkvt", [HID, 512], bf16, isOutput=False)
    p_wot = nc.declare_dram_parameter("wot", [256, HID], bf16, isOutput=False)
    # gate/up: column groups [g0 g1 g2 | u0 u1 u2 | g3 g4 g5 | u3 u4 u5]
    p_wgut = nc.declare_dram_parameter("wgut", [NG, HID, 384], bf16, isOutput=False)
    p_wdt = nc.declare_dram_parameter("wdt", [DG, MI, 256], bf16, isOutput=False)
    p_ln1 = nc.declare_dram_parameter("ln1w", [HID, 1], f32, isOutput=False)
    p_ln2 = nc.declare_dram_parameter("ln2w", [HID, 1], f32, isOutput=False)
    p_cosq = nc.declare_dram_parameter("cosq", [T, D], f32, isOutput=False)
    p_sinq = nc.declare_dram_parameter("sinq", [T, D], f32, isOutput=False)
    p_cosk = nc.declare_dram_parameter("cosk", [T, D], f32, isOutput=False)
    p_sink = nc.declare_dram_parameter("sink", [T, D], f32, isOutput=False)
    p_mask = nc.declare_dram_parameter("mask", [T, T], f32, isOutput=False)

    p_outh = nc.declare_dram_parameter("out_hid", [HID // NC, T], bf16, isOutput=True)
    p_outk = nc.declare_dram_parameter("out_k", [T, D], f32, isOutput=True)
    p_outv = nc.declare_dram_parameter("out_v", [T, D], f32, isOutput=True)

    groups = [list(range(NC))]

    with tile.TileContext(nc) as tc:
        with (
            tc.tile_pool(name="dram", bufs=1, space="DRAM") as dram,
            tc.tile_pool(name="const", bufs=1) as const,
            tc.tile_pool(name="xpool", bufs=1) as xpool,
            tc.tile_pool(name="wstream", bufs=8) as wstream,
            tc.tile_pool(name="scratch", bufs=2) as scratch,
            tc.tile_pool(name="small", bufs=8) as small,
            tc.tile_pool(name="attn", bufs=1) as attn,
            tc.tile_pool(name="psacc", bufs=2, space="PSUM") as psacc,
            tc.tile_pool(name="psgrp", bufs=3, space="PSUM") as psgrp,
            tc.tile_pool(name="pstp", bufs=2, space="PSUM") as pstp,
        ):
            # collective bounce buffers (tracked DRAM tiles)
            cc1_sizes = [1024, 1024]
            cc1_ins = [dram.tile([cc1_sizes[i], T], bf16, name=f"cc1_in{i}")
                       for i in range(2)]
            cc1_outs = [dram.tile([cc1_sizes[i], T], bf16, addr_space="Shared",
                                  name=f"cc1_out{i}") for i in range(2)]
            cc2_sizes = [1024, 512, 512]
            cc2_ins = [dram.tile([cc2_sizes[i], T], bf16, name=f"cc2_in{i}")
                       for i in range(3)]
            cc2_out = dram.tile([HID // NC, T], bf16)
            rinv_dram = dram.tile([T, 1], f32)

            x_prime = xpool.tile([128, KT, T], f32, tag="xres", name="x_sb")
            nc.sync.dma_start(out=x_prime[:, 0], in_=p_x[ts(0, 128), :])
            wqkv_sb = const.tile([128, KT, 512], bf16)
            nc.sync.dma_start(out=wqkv_sb[:, 0], in_=p_wqkvt[ts(0, 128), :])

            ident = const.tile([128, 128], bf16)
            make_identity(nc, ident[:])
            eps_col = const.tile([128, 1], f32)
            nc.vector.memset(eps_col[:], EPS)

            # per-feature norm weights: (2048,1) -> (128, 16) column tiles
            ln1_sb = const.tile([128, KT], f32)
            nc.sync.dma_start(out=ln1_sb[:], in_=p_ln1[:].rearrange("(k p) o -> p (k o)", p=128))
            ln2_sb = const.tile([128, KT], f32)
            nc.sync.dma_start(out=ln2_sb[:], in_=p_ln2[:].rearrange("(k p) o -> p (k o)", p=128))

            # ---- phase A: stream x, LN1 stats, xw = x*ln1_w (bf16) ----
            xw_sb = xpool.tile([128, KT, T], bf16, tag="xw")
            x_sb = x_prime
            ss_acc = attn.tile([128, T], f32, tag="ssacc")
            dma_engs = [nc.sync, nc.scalar, nc.gpsimd]
            for k in range(KT):
                if k > 0:
                    dma_engs[k % 3].dma_start(out=x_sb[:, k], in_=p_x[ts(k, 128), :])
                x_t = x_sb[:, k]
                xsq = scratch.tile([128, T], bf16, tag="xsq")
                nc.scalar.activation(xsq[:], x_t, AF.Square)
                if k == 0:
                    nc.vector.tensor_copy(ss_acc[:], xsq[:])
                else:
                    nc.vector.tensor_tensor(ss_acc[:], ss_acc[:], xsq[:], op=ALU.add)
                nc.vector.tensor_tensor(xw_sb[:, k], x_t, ln1_sb[:, ds(k, 1)].to_broadcast((128, T)), op=ALU.mult)

            # rinv broadcast (128, T): 1/sqrt(sum/HID + eps) on every partition
            rinv_bc = attn.tile([128, T], f32, tag="rbc")
            nc.gpsimd.partition_all_reduce(rinv_bc[:], ss_acc[:], channels=128,
                                           reduce_op=bass_isa.ReduceOp.add)
            nc.scalar.activation(rinv_bc[:], rinv_bc[:], AF.Sqrt,
                                 bias=eps_col[:], scale=1.0 / HID)
            nc.vector.reciprocal(rinv_bc[:], rinv_bc[:])
            # token-major (128, TT) copy via DRAM bounce (for the v output)
            nc.sync.dma_start(out=rinv_dram[:], in_=rinv_bc[0:1, :])
            rinv_tm = small.tile([128, TT], f32, tag="rtm")
            nc.sync.dma_start(out=rinv_tm[:],
                              in_=rinv_dram[:].rearrange("(t p) o -> p (t o)", p=128))

            # ---- phase B: QKV (token-major) + head RMS + RoPE ----
            for k in range(1, KT):
                dma_engs[(k + 1) % 3].dma_start(out=wqkv_sb[:, k], in_=p_wqkvt[ts(k, 128), :])

            # RoPE tables + attention mask, token-tiled
            cosq_sb = const.tile([128, TT, D], f32)
            sinq_sb = const.tile([128, TT, D], f32)
            cosk_sb = const.tile([128, TT, D], f32)
            sink_sb = const.tile([128, TT, D], f32)
            mask_sb = const.tile([128, TT, T], f32)
            for t in range(TT):
                nc.sync.dma_start(out=cosq_sb[:, t], in_=p_cosq[ts(t, 128), :])
                nc.sync.dma_start(out=sinq_sb[:, t], in_=p_sinq[ts(t, 128), :])
                nc.sync.dma_start(out=cosk_sb[:, t], in_=p_cosk[ts(t, 128), :])
                nc.sync.dma_start(out=sink_sb[:, t], in_=p_sink[ts(t, 128), :])
                nc.sync.dma_start(out=mask_sb[:, t], in_=p_mask[ts(t, 128), :])


            q_fm = attn.tile([128, QH, T], bf16)   # q feature-major per head
            rq_all = {}
            k_fm = attn.tile([128, T], bf16)
            v_tm = attn.tile([128, TT, D], bf16)   # v token-major (unscaled)

            def rms_rope(dst_bf, src_ap, cos_t, sin_t, tag, dst_f32=None,
                         defer_scale=False):
                # src_ap: (128 tokens, 128 dims) PSUM f32
                sq = scratch.tile([128, D], bf16, tag="rsq")
                ssq = small.tile([128, 1], f32, tag=tag + "ss")
                nc.scalar.activation(sq[:], src_ap, AF.Square, accum_out=ssq[:])
                rr = small.tile([128, 1], f32, tag=tag + "rr", name=f"rr_{tag}")
                nc.scalar.activation(rr[:], ssq[:], AF.Sqrt, bias=eps_col[:], scale=1.0 / D)
                nc.vector.reciprocal(rr[:], rr[:])
                t1 = scratch.tile([128, D], f32, tag="rt1")
                t2 = scratch.tile([128, D], f32, tag="rt2")
                if defer_scale:
                    # rope WITHOUT the rms scale (applied later via exp scale);
                    # runs concurrently with the ssq/sqrt/recip chain
                    nc.vector.tensor_tensor(t1[:], src_ap, cos_t, op=ALU.mult)
                    nc.vector.tensor_tensor(t2[:, 0:64], src_ap[:, 64:128],
                                            sin_t[:, 0:64], op=ALU.mult)
                    nc.vector.tensor_tensor(t2[:, 64:128], src_ap[:, 0:64],
                                            sin_t[:, 64:128], op=ALU.mult)
                else:
                    nc.vector.scalar_tensor_tensor(
                        t1[:], src_ap, rr[:], cos_t, op0=ALU.mult, op1=ALU.mult)
                    nc.vector.scalar_tensor_tensor(
                        t2[:, 0:64], src_ap[:, 64:128], rr[:], sin_t[:, 0:64],
                        op0=ALU.mult, op1=ALU.mult)
                    nc.vector.scalar_tensor_tensor(
                        t2[:, 64:128], src_ap[:, 0:64], rr[:], sin_t[:, 64:128],
                        op0=ALU.mult, op1=ALU.mult)
                if dst_f32 is not None:
                    nc.vector.tensor_tensor(dst_f32, t1[:], t2[:], op=ALU.add)
                    nc.vector.tensor_copy(dst_bf, dst_f32)
                else:
                    nc.vector.tensor_tensor(dst_bf, t1[:], t2[:], op=ALU.add)
                return rr

            for t in range(TT):
                qkv_ps = psgrp.tile([128, 512], f32, tag="grp")
                for k in range(KT):
                    nc.tensor.matmul(qkv_ps[:], xw_sb[:, k, ts(t, 128)], wqkv_sb[:, k],
                                     start=(k == 0), stop=(k == KT - 1))
                q_ps = qkv_ps
                kv_ps = qkv_ps[:, 256:512]

                # q heads: RoPE (rms scale deferred to the exp) -> feature-major
                for h in range(QH):
                    qr = scratch.tile([128, D], bf16, tag="qr")
                    rq = rms_rope(qr[:], qkv_ps[:, ds(h * 128, 128)],
                                  cosq_sb[:, t], sinq_sb[:, t], f"q{h}",
                                  defer_scale=True)
                    rq_all[(t, h)] = rq
                    qt_ps = pstp.tile([128, 128], bf16, tag="tps")
                    nc.tensor.transpose(qt_ps[:], qr[:], ident[:])
                    nc.vector.tensor_copy(q_fm[:, h, ts(t, 128)], qt_ps[:])

                # k head: RMS + RoPE -> out_k (f32) + feature-major (bf16)
                krb = scratch.tile([128, D], bf16, tag="krb")
                krf = scratch.tile([128, D], f32, tag="krf")
                rms_rope(krb[:], qkv_ps[:, 256:384], cosk_sb[:, t], sink_sb[:, t],
                         "k", dst_f32=krf[:])
                nc.sync.dma_start(out=p_outk[ts(t, 128), :], in_=krf[:])
                kt_ps = pstp.tile([128, 128], bf16, tag="tps")
                nc.tensor.transpose(kt_ps[:], krb[:], ident[:])
                nc.vector.tensor_copy(k_fm[:, ts(t, 128)], kt_ps[:])

                # v: unscaled bf16 for attention; ln1-scaled f32 for output
                nc.vector.tensor_copy(v_tm[:, t], qkv_ps[:, 384:512])
                vout = scratch.tile([128, D], f32, tag="vout")
                nc.vector.tensor_tensor(vout[:], qkv_ps[:, 384:512],
                                        rinv_tm[:, ds(t, 1)].to_broadcast((128, D)), op=ALU.mult)
                nc.sync.dma_start(out=p_outv[ts(t, 128), :], in_=vout[:])

            # ---- phase D: attention, heads interleaved, causal-trimmed ----
            # (masked scores are exactly exp(-1e4)=0 in f32, so computing only
            #  the visible lower-triangular tile strip is exact)
            o_fm = attn.tile([128, QH, T], bf16)
            pT_sbs = [attn.tile([128, TT, T], bf16, name=f"pT{h}", tag=f"pT{h}")
                      for h in range(QH)]
            for h in range(QH):
                nc.vector.memset(pT_sbs[h][:], 0.0)
            for t in range(TT):
                W = (t + 1) * 128
                for h in range(QH):
                    s_ps = psacc.tile([128, T], f32, tag="acc")
                    nc.tensor.matmul(s_ps[:, 0:W], q_fm[:, h, ts(t, 128)],
                                     k_fm[:, 0:W], start=True, stop=True)
                    s_sb = scratch.tile([128, T], f32, tag="ssb")
                    nc.vector.scalar_tensor_tensor(
                        s_sb[:, 0:W], s_ps[:, 0:W], SCALE, mask_sb[:, t, 0:W],
                        op0=ALU.mult, op1=ALU.add)
                    p_sb = scratch.tile([128, T], f32, tag="psb")
                    rowsum = small.tile([128, 1], f32, tag="rsum")
                    nc.scalar.activation(p_sb[:, 0:W], s_sb[:, 0:W], AF.Exp,
                                         scale=rq_all[(t, h)][:],
                                         accum_out=rowsum[:])
                    nc.vector.reciprocal(rowsum[:], rowsum[:])
                    pn = scratch.tile([128, T], bf16, tag="pn")
                    nc.vector.scalar_tensor_tensor(
                        pn[:, 0:W], p_sb[:, 0:W], rowsum[:], rinv_bc[:, 0:W],
                        op0=ALU.mult, op1=ALU.mult)
                    for st in range(t + 1):
                        pt_ps = pstp.tile([128, 128], bf16, tag="tps")
                        nc.tensor.transpose(pt_ps[:], pn[:, ts(st, 128)], ident[:])
                        nc.vector.tensor_copy(pT_sbs[h][:, st, ts(t, 128)], pt_ps[:])
            for h in range(QH):
                o_ps = psacc.tile([128, T], f32, tag="acc")
                for st in range(TT):
                    nc.tensor.matmul(o_ps[:], v_tm[:, st], pT_sbs[h][:, st],
                                     start=(st == 0), stop=(st == TT - 1))
                nc.vector.tensor_copy(o_fm[:, h], o_ps[:])

            # ---- phase E: o-proj partials -> AllReduce ----
            wo_sb = const.tile([128, 2, HID], bf16)
            for k in range(2):
                nc.sync.dma_start(out=wo_sb[:, k], in_=p_wot[ts(k, 128), :])
            cc1_mranges = [(0, 8), (8, 16)]
            for q, (m0, m1) in enumerate(cc1_mranges):
                for m in range(m0, m1):
                    ao_ps = psacc.tile([128, T], f32, tag="acc")
                    for k in range(2):
                        nc.tensor.matmul(ao_ps[:], wo_sb[:, k, ts(m, 128)], o_fm[:, k],
                                         start=(k == 0), stop=(k == 1))
                    ao_sb = scratch.tile([128, T], bf16, tag="aosb")
                    nc.any.tensor_copy(ao_sb[:], ao_ps[:])
                    nc.sync.dma_start(out=cc1_ins[q][ts(m - m0, 128), :], in_=ao_sb[:])
                nc.gpsimd.collective_compute(
                    "AllReduce", ALU.add,
                    ins=[cc1_ins[q][:]],
                    outs=[cc1_outs[q][:]],
                    replica_groups=groups)

            # ---- phase F: residual + LN2 + MLP + ReduceScatter ----
            hm_sb = xpool.tile([128, KT, T], f32, tag="hm")
            hw2_sb = xpool.tile([128, KT, T], bf16, tag="xw")   # reuse xw slot
            ss2_acc = attn.tile([128, T], f32, tag="ss2acc")
            for k in range(KT):
                x2 = x_sb[:, k]
                ao = scratch.tile([128, T], bf16, tag="aoin")
                nc.gpsimd.dma_start(out=ao[:], in_=cc1_outs[k // 8][ts(k % 8, 128), :])
                nc.vector.tensor_tensor(hm_sb[:, k], x2, ao[:], op=ALU.add)
                xsq2 = scratch.tile([128, T], bf16, tag="xsq")
                nc.scalar.activation(xsq2[:], hm_sb[:, k], AF.Square)
                if k == 0:
                    nc.vector.tensor_copy(ss2_acc[:], xsq2[:])
                else:
                    nc.vector.tensor_tensor(ss2_acc[:], ss2_acc[:], xsq2[:], op=ALU.add)
                nc.vector.tensor_tensor(hw2_sb[:, k], hm_sb[:, k],
                                        ln2_sb[:, ds(k, 1)].to_broadcast((128, T)),
                                        op=ALU.mult)

            rinv2_bc = attn.tile([128, T], f32, tag="r2bc")
            nc.gpsimd.partition_all_reduce(rinv2_bc[:], ss2_acc[:], channels=128,
                                           reduce_op=bass_isa.ReduceOp.add)
            nc.scalar.activation(rinv2_bc[:], rinv2_bc[:], AF.Sqrt,
                                 bias=eps_col[:], scale=1.0 / HID)
            nc.vector.reciprocal(rinv2_bc[:], rinv2_bc[:])

            # gate/up: 4 groups of 3 m-tiles (g g g | u u u | g g g | u u u)
            act_sb = attn.tile([128, MI // 128, T], bf16, tag="act")
            gate_tiles = {}
            for g in range(NG):
                gu_ps = [psgrp.tile([128, T], f32, tag="grp", name=f"gu_ps{g}_{i}") for i in range(3)]
                for k in range(KT):
                    wtile = wstream.tile([128, 384], bf16, tag="wgu")
                    nc.sync.dma_start(out=wtile[:], in_=p_wgut[g, ts(k, 128), :])
                    for mi in range(3):
                        nc.tensor.matmul(gu_ps[mi][:], wtile[:, ds(mi * 128, 128)],
                                         hw2_sb[:, k],
                                         start=(k == 0), stop=(k == KT - 1))
                is_gate = (g % 2 == 0)
                base = (g // 2) * 3
                for mi in range(3):
                    pre = scratch.tile([128, T], f32, tag="gupre")
                    nc.vector.tensor_tensor(pre[:], gu_ps[mi][:], rinv2_bc[:],
                                            op=ALU.mult)
                    if is_gate:
                        gt = attn.tile([128, T], f32, tag=f"gate{mi}")
                        nc.scalar.activation(gt[:], pre[:], AF.Silu)
                        gate_tiles[base + mi] = gt
                    else:
                        nc.vector.tensor_tensor(act_sb[:, base + mi],
                                                gate_tiles[base + mi][:], pre[:],
                                                op=ALU.mult)

            # down-proj: 8 groups of 2 m-tiles + residual/8 -> cc2_in
            # (RS chunk emitted after each half's groups)
            for g in range(DG):
                d_ps = [psgrp.tile([128, T], f32, tag="grp", name=f"d_ps{g}_{i}") for i in range(2)]
                for k in range(MI // 128):
                    wtile = wstream.tile([128, 256], bf16, tag="wd")
                    nc.sync.dma_start(out=wtile[:], in_=p_wdt[g, ts(k, 128), :])
                    for mi in range(2):
                        nc.tensor.matmul(d_ps[mi][:], wtile[:, ds(mi * 128, 128)],
                                         act_sb[:, k],
                                         start=(k == 0), stop=(k == MI // 128 - 1))
                for mi in range(2):
                    m = g * 2 + mi
                    fin = scratch.tile([128, T], bf16, tag="fin")
                    nc.vector.scalar_tensor_tensor(
                        fin[:], hm_sb[:, m], 1.0 / NC, d_ps[mi][:],
                        op0=ALU.mult, op1=ALU.add)
                    mq = 0 if m < 8 else (1 if m < 12 else 2)
                    nc.gpsimd.dma_start(out=cc2_ins[mq][ts(m - (0, 8, 12)[mq], 128), :], in_=fin[:])
                if g in (3, 5, 7):
                    q = (3, 5, 7).index(g)
                    out_off, out_n = ((0, 128), (128, 64), (192, 64))[q]
                    nc.gpsimd.collective_compute(
                        "ReduceScatter", ALU.add,
                        ins=[cc2_ins[q][:]],
                        outs=[cc2_out[out_off:out_off + out_n, :]],
                        replica_groups=groups)

            nc.sync.dma_start(out=p_outh[:], in_=cc2_out[:])

    nc.compile()
    return nc


def _get_nc():
    if "nc" not in _CACHE:
        _patch_walrus_flags()
        _CACHE["nc"] = _build()
    return _CACHE["nc"]


def _shard_inputs(hidden_conv, cos, sin, attn_mask, wq, wk, wv, wo,
                  ln1_w, ln2_w, qn_w, kn_w, w_gate_up, w_down):
    import ml_dtypes
    f = np.float32
    bf = ml_dtypes.bfloat16
    x_fm = np.ascontiguousarray(np.asarray(hidden_conv, f)[0, :, 0, :])   # (2048, 512)
    cos2 = np.asarray(cos, f)[0]
    sin2 = np.asarray(sin, f)[0]
    mask = np.ascontiguousarray(np.asarray(attn_mask, f)[0, 0, :, :T])    # (512, 512)
    qn = np.asarray(qn_w, f); kn = np.asarray(kn_w, f)
    ln1 = np.ascontiguousarray(np.asarray(ln1_w, f).reshape(HID, 1))
    ln2 = np.ascontiguousarray(np.asarray(ln2_w, f).reshape(HID, 1))
    wq = np.asarray(wq, f); wk = np.asarray(wk, f); wv = np.asarray(wv, f)
    wo = np.asarray(wo, f); wgu = np.asarray(w_gate_up, f); wd = np.asarray(w_down, f)

    def rope_tables(w):
        cosw = cos2 * w[None, :]
        sinw = np.concatenate(
            [-sin2[:, 0:64] * w[None, 64:128], sin2[:, 64:128] * w[None, 0:64]], axis=1)
        return np.ascontiguousarray(cosw), np.ascontiguousarray(sinw)

    cosq, sinq = rope_tables(qn)
    cosk, sink = rope_tables(kn)

    in_maps = []
    for c in range(NC):
        wqkvt = np.ascontiguousarray(
            np.concatenate([wq[c * 256:(c + 1) * 256], wk[c * 128:(c + 1) * 128],
                            wv[c * 128:(c + 1) * 128]], 0).T.astype(bf))
        wot = np.ascontiguousarray(wo[:, c * 256:(c + 1) * 256].T.astype(bf))
        # (2048, 1536) -> NG groups of 384 cols: [g0 g1 g2 | u0 u1 u2 | g3.. ]
        wg = wgu[c * MI:(c + 1) * MI].T            # (2048, 768) gate
        wu = wgu[INT + c * MI:INT + (c + 1) * MI].T
        wgut = np.stack([
            wg[:, 0:384], wu[:, 0:384], wg[:, 384:768], wu[:, 384:768]], 0)
        wgut = np.ascontiguousarray(wgut.astype(bf))           # (4, 2048, 384)
        wdt = wd[:, c * MI:(c + 1) * MI].T         # (768, 2048)
        wdt = np.ascontiguousarray(
            wdt.reshape(MI, DG, 256).transpose(1, 0, 2).astype(bf))  # (8, 768, 256)
        in_maps.append(dict(
            x=x_fm, wqkvt=wqkvt, wot=wot, wgut=wgut, wdt=wdt,
            ln1w=ln1, ln2w=ln2, cosq=cosq, sinq=sinq, cosk=cosk, sink=sink,
            mask=mask))
    return in_maps


def kernel(hidden_conv, cos, sin, update_mask, attn_mask, k_cache, v_cache,
           wq, wk, wv, wo, ln1_w, ln2_w, qn_w, kn_w, w_gate_up, w_down):
    import os
    from concourse.bass_utils import run_bass_kernel_spmd

    in_maps = _shard_inputs(hidden_conv, cos, sin, attn_mask, wq, wk, wv, wo,
                            ln1_w, ln2_w, qn_w, kn_w, w_gate_up, w_down)
    nc = _get_nc()
    trace = bool(os.environ.get("KERNEL_TRACE"))
    res = run_bass_kernel_spmd(nc, in_maps, core_ids=list(range(NC)), trace=trace)
    _CACHE["last_res"] = res
    results = res.results

    hid = np.empty((HID, T), np.float32)
    for c in range(NC):
        oh = np.asarray(results[c]["out_hid"], np.float32)
        hid[c * 128:(c + 1) * 128] = oh[0:128]
        hid[1024 + c * 64:1024 + (c + 1) * 64] = oh[128:192]
        hid[1536 + c * 64:1536 + (c + 1) * 64] = oh[192:256]
    hid = np.ascontiguousarray(hid.reshape(1, HID, 1, T))
    kf = np.zeros((1, HKV, S, D), np.float32)
    vf = np.zeros((1, HKV, S, D), np.float32)
    for c in range(NC):
        kf[0, c, :T] = results[c]["out_k"]
        vf[0, c, :T] = results[c]["out_v"]
    return hid, kf, vf


# revision 31
# speedup vs baseline: 1.0302x; 1.0005x over previous
"""Trainium2 Bass kernel for nn_ANEPrefillLayer (dense transformer prefill layer).

Tensor-parallel over 8 NeuronCores:
  - q heads: 2 per core; kv head: 1 per core
  - o-proj column-sharded -> AllReduce of attention-output partials
  - MLP intermediate sharded (768 per core) -> ReduceScatter of (mlp + hm/8),
    each core emitting a 256-row feature shard of the final hidden state.
Matmul operands are cast to bf16 on chip (weights cast on host), fp32
accumulation in PSUM.  Activations kept feature-major (features on
partitions); q/k/v projections are produced token-major directly by using the
activation tiles as the stationary operand.
"""
import numpy as np

HID, H, HKV, D, INT, S, T = 2048, 16, 8, 128, 6144, 4096, 512
EPS = 1e-6
SCALE = 1.0 / D**0.5
NC = 8
KT = HID // 128          # 16 k-tiles over hidden
TT = T // 128            # 4 token tiles
QH = H // NC             # 2 q heads per core
MI = INT // NC           # 768 intermediate per core
NG = 4                   # gate/up m-groups (3 m-tiles each)
DG = 8                   # down-proj m-groups (2 m-tiles each)

_CACHE = {}


def _patch_walrus_flags():
    # pair LDWEIGHTS with MATMULs (the default disables the optimization)
    return


def _build():
    import concourse.bass as bass
    import concourse.bass_isa as bass_isa
    import concourse.mybir as mybir
    import concourse.tile as tile
    from concourse.masks import make_identity
    from concourse.bass import ds, ts

    f32 = mybir.dt.float32
    bf16 = mybir.dt.bfloat16
    AF = mybir.ActivationFunctionType
    ALU = mybir.AluOpType

    from concourse import bacc
    nc = bacc.Bacc()

    # ---- DRAM parameters (per-core shards; same graph on all cores) ----
    p_x = nc.declare_dram_parameter("x", [HID, T], f32, isOutput=False)
    p_wqkvt = nc.declare_dram_parameter("wqkvt", [HID, 512], bf16, isOutput=False)
    p_wot = nc.declare_dram_parameter("wot", [256, HID], bf16, isOutput=False)
    # gate/up: column groups [g0 g1 g2 | u0 u1 u2 | g3 g4 g5 | u3 u4 u5]
    p_wgut = nc.declare_dram_parameter("wgut", [NG, HID, 384], bf16, isOutput=False)
    p_wdt = nc.declare_dram_parameter("wdt", [DG, MI, 256], bf16, isOutput=False)
    p_ln1 = nc.declare_dram_parameter("ln1w", [HID, 1], f32, isOutput=False)
    p_ln2 = nc.declare_dram_parameter("ln2w", [HID, 1], f32, isOutput=False)
    p_cosq = nc.declare_dram_parameter("cosq", [T, D], f32, isOutput=False)
    p_sinq = nc.declare_dram_parameter("sinq", [T, D], f32, isOutput=False)
    p_cosk = nc.declare_dram_parameter("cosk", [T, D], f32, isOutput=False)
    p_sink = nc.declare_dram_parameter("sink", [T, D], f32, isOutput=False)
    p_mask = nc.declare_dram_parameter("mask", [T, T], f32, isOutput=False)

    p_outh = nc.declare_dram_parameter("out_hid", [HID // NC, T], bf16, isOutput=True)
    p_outk = nc.declare_dram_parameter("out_k", [T, D], f32, isOutput=True)
    p_outv = nc.declare_dram_parameter("out_v", [T, D], f32, isOutput=True)

    groups = [list(range(NC))]

    with tile.TileContext(nc) as tc:
        with (
            tc.tile_pool(name="dram", bufs=1, space="DRAM") as dram,
            tc.tile_pool(name="const", bufs=1) as const,
            tc.tile_pool(name="xpool", bufs=1) as xpool,
            tc.tile_pool(name="wstream", bufs=8) as wstream,
            tc.tile_pool(name="scratch", bufs=2) as scratch,
            tc.tile_pool(name="small", bufs=8) as small,
            tc.tile_pool(name="attn", bufs=1) as attn,
            tc.tile_pool(name="psacc", bufs=2, space="PSUM") as psacc,
            tc.tile_pool(name="psgrp", bufs=3, space="PSUM") as psgrp,
            tc.tile_pool(name="pstp", bufs=2, space="PSUM") as pstp,
        ):
            # collective bounce buffers (tracked DRAM tiles)
            cc1_sizes = [1024, 1024]
            cc1_ins = [dram.tile([cc1_sizes[i], T], bf16, name=f"cc1_in{i}")
                       for i in range(2)]
            cc1_outs = [dram.tile([cc1_sizes[i], T], bf16, addr_space="Shared",
                                  name=f"cc1_out{i}") for i in range(2)]
            cc2_ins = [dram.tile([512, T], bf16, name=f"cc2_in{i}")
                       for i in range(4)]
            cc2_out = dram.tile([HID // NC, T], bf16)
            rinv_dram = dram.tile([T, 1], f32)

            x_prime = xpool.tile([128, KT, T], f32, tag="xres", name="x_sb")
            nc.sync.dma_start(out=x_prime[:, 0], in_=p_x[ts(0, 128), :])
            wqkv_sb = const.tile([128, KT, 512], bf16)
            nc.sync.dma_start(out=wqkv_sb[:, 0], in_=p_wqkvt[ts(0, 128), :])

            ident = const.tile([128, 128], bf16)
            make_identity(nc, ident[:])
            eps_col = const.tile([128, 1], f32)
            nc.vector.memset(eps_col[:], EPS)

            # per-feature norm weights: (2048,1) -> (128, 16) column tiles
            ln1_sb = const.tile([128, KT], f32)
            nc.sync.dma_start(out=ln1_sb[:], in_=p_ln1[:].rearrange("(k p) o -> p (k o)", p=128))
            ln2_sb = const.tile([128, KT], f32)
            nc.sync.dma_start(out=ln2_sb[:], in_=p_ln2[:].rearrange("(k p) o -> p (k o)", p=128))

            # ---- phase A: stream x, LN1 stats, xw = x*ln1_w (bf16) ----
            xw_sb = xpool.tile([128, KT, T], bf16, tag="xw")
            x_sb = x_prime
            ss_acc = attn.tile([128, T], f32, tag="ssacc")
            dma_engs = [nc.sync, nc.scalar, nc.gpsimd]
            for k in range(KT):
                if k > 0:
                    dma_engs[k % 3].dma_start(out=x_sb[:, k], in_=p_x[ts(k, 128), :])
                x_t = x_sb[:, k]
                xsq = scratch.tile([128, T], bf16, tag="xsq")
                nc.scalar.activation(xsq[:], x_t, AF.Square)
                if k == 0:
                    nc.vector.tensor_copy(ss_acc[:], xsq[:])
                else:
                    nc.vector.tensor_tensor(ss_acc[:], ss_acc[:], xsq[:], op=ALU.add)
                nc.vector.tensor_tensor(xw_sb[:, k], x_t, ln1_sb[:, ds(k, 1)].to_broadcast((128, T)), op=ALU.mult)

            # rinv broadcast (128, T): 1/sqrt(sum/HID + eps) on every partition
            rinv_bc = attn.tile([128, T], f32, tag="rbc")
            nc.gpsimd.partition_all_reduce(rinv_bc[:], ss_acc[:], channels=128,
                                           reduce_op=bass_isa.ReduceOp.add)
            nc.scalar.activation(rinv_bc[:], rinv_bc[:], AF.Sqrt,
                                 bias=eps_col[:], scale=1.0 / HID)
            nc.vector.reciprocal(rinv_bc[:], rinv_bc[:])
            # token-major (128, TT) copy via DRAM bounce (for the v output)
            nc.sync.dma_start(out=rinv_dram[:], in_=rinv_bc[0:1, :])
            rinv_tm = small.tile([128, TT], f32, tag="rtm")
            nc.sync.dma_start(out=rinv_tm[:],
                              in_=rinv_dram[:].rearrange("(t p) o -> p (t o)", p=128))

            # ---- phase B: QKV (token-major) + head RMS + RoPE ----
            for k in range(1, KT):
                dma_engs[(k + 1) % 3].dma_start(out=wqkv_sb[:, k], in_=p_wqkvt[ts(k, 128), :])

            # RoPE tables + attention mask, token-tiled
            cosq_sb = const.tile([128, TT, D], f32)
            sinq_sb = const.tile([128, TT, D], f32)
            cosk_sb = const.tile([128, TT, D], f32)
            sink_sb = const.tile([128, TT, D], f32)
            mask_sb = const.tile([128, TT, T], f32)
            for t in range(TT):
                nc.sync.dma_start(out=cosq_sb[:, t], in_=p_cosq[ts(t, 128), :])
                nc.sync.dma_start(out=sinq_sb[:, t], in_=p_sinq[ts(t, 128), :])
                nc.sync.dma_start(out=cosk_sb[:, t], in_=p_cosk[ts(t, 128), :])
                nc.sync.dma_start(out=sink_sb[:, t], in_=p_sink[ts(t, 128), :])
                nc.sync.dma_start(out=mask_sb[:, t], in_=p_mask[ts(t, 128), :])


            q_fm = attn.tile([128, QH, T], bf16)   # q feature-major per head
            rq_all = {}
            k_fm = attn.tile([128, T], bf16)
            v_tm = attn.tile([128, TT, D], bf16)   # v token-major (unscaled)

            def rms_rope(dst_bf, src_ap, cos_t, sin_t, tag, dst_f32=None,
                         defer_scale=False):
                # src_ap: (128 tokens, 128 dims) PSUM f32
                sq = scratch.tile([128, D], bf16, tag="rsq")
                ssq = small.tile([128, 1], f32, tag=tag + "ss")
                nc.scalar.activation(sq[:], src_ap, AF.Square, accum_out=ssq[:])
                rr = small.tile([128, 1], f32, tag=tag + "rr", name=f"rr_{tag}")
                nc.scalar.activation(rr[:], ssq[:], AF.Sqrt, bias=eps_col[:], scale=1.0 / D)
                nc.vector.reciprocal(rr[:], rr[:])
                t1 = scratch.tile([128, D], f32, tag="rt1")
                t2 = scratch.tile([128, D], f32, tag="rt2")
                if defer_scale:
                    # rope WITHOUT the rms scale (applied later via exp scale);
                    # runs concurrently with the ssq/sqrt/recip chain
                    nc.vector.tensor_tensor(t1[:], src_ap, cos_t, op=ALU.mult)
                    nc.vector.tensor_tensor(t2[:, 0:64], src_ap[:, 64:128],
                                            sin_t[:, 0:64], op=ALU.mult)
                    nc.vector.tensor_tensor(t2[:, 64:128], src_ap[:, 0:64],
                                            sin_t[:, 64:128], op=ALU.mult)
                else:
                    nc.vector.scalar_tensor_tensor(
                        t1[:], src_ap, rr[:], cos_t, op0=ALU.mult, op1=ALU.mult)
                    nc.vector.scalar_tensor_tensor(
                        t2[:, 0:64], src_ap[:, 64:128], rr[:], sin_t[:, 0:64],
                        op0=ALU.mult, op1=ALU.mult)
                    nc.vector.scalar_tensor_tensor(
                        t2[:, 64:128], src_ap[:, 0:64], rr[:], sin_t[:, 64:128],
                        op0=ALU.mult, op1=ALU.mult)
                if dst_f32 is not None:
                    nc.vector.tensor_tensor(dst_f32, t1[:], t2[:], op=ALU.add)
                    nc.vector.tensor_copy(dst_bf, dst_f32)
                else:
                    nc.vector.tensor_tensor(dst_bf, t1[:], t2[:], op=ALU.add)
                return rr

            for t in range(TT):
                qkv_ps = psgrp.tile([128, 512], f32, tag="grp")
                for k in range(KT):
                    nc.tensor.matmul(qkv_ps[:], xw_sb[:, k, ts(t, 128)], wqkv_sb[:, k],
                                     start=(k == 0), stop=(k == KT - 1))
                q_ps = qkv_ps
                kv_ps = qkv_ps[:, 256:512]

                # q heads: RoPE (rms scale deferred to the exp) -> feature-major
                for h in range(QH):
                    qr = scratch.tile([128, D], bf16, tag="qr")
                    rq = rms_rope(qr[:], qkv_ps[:, ds(h * 128, 128)],
                                  cosq_sb[:, t], sinq_sb[:, t], f"q{h}",
                                  defer_scale=True)
                    rq_all[(t, h)] = rq
                    qt_ps = pstp.tile([128, 128], bf16, tag="tps")
                    nc.tensor.transpose(qt_ps[:], qr[:], ident[:])
                    nc.vector.tensor_copy(q_fm[:, h, ts(t, 128)], qt_ps[:])

                # k head: RMS + RoPE -> out_k (f32) + feature-major (bf16)
                krb = scratch.tile([128, D], bf16, tag="krb")
                krf = scratch.tile([128, D], f32, tag="krf")
                rms_rope(krb[:], qkv_ps[:, 256:384], cosk_sb[:, t], sink_sb[:, t],
                         "k", dst_f32=krf[:])
                nc.sync.dma_start(out=p_outk[ts(t, 128), :], in_=krf[:])
                kt_ps = pstp.tile([128, 128], bf16, tag="tps")
                nc.tensor.transpose(kt_ps[:], krb[:], ident[:])
                nc.vector.tensor_copy(k_fm[:, ts(t, 128)], kt_ps[:])

                # v: unscaled bf16 for attention; ln1-scaled f32 for output
                nc.vector.tensor_copy(v_tm[:, t], qkv_ps[:, 384:512])
                vout = scratch.tile([128, D], f32, tag="vout")
                nc.vector.tensor_tensor(vout[:], qkv_ps[:, 384:512],
                                        rinv_tm[:, ds(t, 1)].to_broadcast((128, D)), op=ALU.mult)
                nc.sync.dma_start(out=p_outv[ts(t, 128), :], in_=vout[:])

            # ---- phase D: attention, heads interleaved, causal-trimmed ----
            # (masked scores are exactly exp(-1e4)=0 in f32, so computing only
            #  the visible lower-triangular tile strip is exact)
            o_fm = attn.tile([128, QH, T], bf16)
            pT_sbs = [attn.tile([128, TT, T], bf16, name=f"pT{h}", tag=f"pT{h}")
                      for h in range(QH)]
            for h in range(QH):
                nc.vector.memset(pT_sbs[h][:], 0.0)
            for t in range(TT):
                W = (t + 1) * 128
                for h in range(QH):
                    s_ps = psacc.tile([128, T], f32, tag="acc")
                    nc.tensor.matmul(s_ps[:, 0:W], q_fm[:, h, ts(t, 128)],
                                     k_fm[:, 0:W], start=True, stop=True)
                    s_sb = scratch.tile([128, T], f32, tag="ssb")
                    nc.vector.scalar_tensor_tensor(
                        s_sb[:, 0:W], s_ps[:, 0:W], SCALE, mask_sb[:, t, 0:W],
                        op0=ALU.mult, op1=ALU.add)
                    p_sb = scratch.tile([128, T], f32, tag="psb")
                    rowsum = small.tile([128, 1], f32, tag="rsum")
                    nc.scalar.activation(p_sb[:, 0:W], s_sb[:, 0:W], AF.Exp,
                                         scale=rq_all[(t, h)][:],
                                         accum_out=rowsum[:])
                    nc.vector.reciprocal(rowsum[:], rowsum[:])
                    pn = scratch.tile([128, T], bf16, tag="pn")
                    nc.vector.scalar_tensor_tensor(
                        pn[:, 0:W], p_sb[:, 0:W], rowsum[:], rinv_bc[:, 0:W],
                        op0=ALU.mult, op1=ALU.mult)
                    for st in range(t + 1):
                        pt_ps = pstp.tile([128, 128], bf16, tag="tps")
                        nc.tensor.transpose(pt_ps[:], pn[:, ts(st, 128)], ident[:])
                        nc.vector.tensor_copy(pT_sbs[h][:, st, ts(t, 128)], pt_ps[:])
            for h in range(QH):
                o_ps = psacc.tile([128, T], f32, tag="acc")
                for st in range(TT):
                    nc.tensor.matmul(o_ps[:], v_tm[:, st], pT_sbs[h][:, st],
                                     start=(st == 0), stop=(st == TT - 1))
                nc.vector.tensor_copy(o_fm[:, h], o_ps[:])

            # ---- phase E: o-proj partials -> AllReduce ----
            wo_sb = const.tile([128, 2, HID], bf16)
            for k in range(2):
                nc.sync.dma_start(out=wo_sb[:, k], in_=p_wot[ts(k, 128), :])
            cc1_mranges = [(0, 8), (8, 16)]
            for q, (m0, m1) in enumerate(cc1_mranges):
                for m in range(m0, m1):
                    ao_ps = psacc.tile([128, T], f32, tag="acc")
                    for k in range(2):
                        nc.tensor.matmul(ao_ps[:], wo_sb[:, k, ts(m, 128)], o_fm[:, k],
                                         start=(k == 0), stop=(k == 1))
                    ao_sb = scratch.tile([128, T], bf16, tag="aosb")
                    nc.any.tensor_copy(ao_sb[:], ao_ps[:])
                    nc.sync.dma_start(out=cc1_ins[q][ts(m - m0, 128), :], in_=ao_sb[:])
                nc.gpsimd.collective_compute(
                    "AllReduce", ALU.add,
                    ins=[cc1_ins[q][:]],
                    outs=[cc1_outs[q][:]],
                    replica_groups=groups)

            # ---- phase F: residual + LN2 + MLP + ReduceScatter ----
            hm_sb = xpool.tile([128, KT, T], f32, tag="hm")
            hw2_sb = xpool.tile([128, KT, T], bf16, tag="xw")   # reuse xw slot
            ss2_acc = attn.tile([128, T], f32, tag="ss2acc")
            for k in range(KT):
                x2 = x_sb[:, k]
                ao = scratch.tile([128, T], bf16, tag="aoin")
                nc.gpsimd.dma_start(out=ao[:], in_=cc1_outs[k // 8][ts(k % 8, 128), :])
                nc.vector.tensor_tensor(hm_sb[:, k], x2, ao[:], op=ALU.add)
                xsq2 = scratch.tile([128, T], bf16, tag="xsq")
                nc.scalar.activation(xsq2[:], hm_sb[:, k], AF.Square)
                if k == 0:
                    nc.vector.tensor_copy(ss2_acc[:], xsq2[:])
                else:
                    nc.vector.tensor_tensor(ss2_acc[:], ss2_acc[:], xsq2[:], op=ALU.add)
                nc.vector.tensor_tensor(hw2_sb[:, k], hm_sb[:, k],
                                        ln2_sb[:, ds(k, 1)].to_broadcast((128, T)),
                                        op=ALU.mult)

            rinv2_bc = attn.tile([128, T], f32, tag="r2bc")
            nc.gpsimd.partition_all_reduce(rinv2_bc[:], ss2_acc[:], channels=128,
                                           reduce_op=bass_isa.ReduceOp.add)
            nc.scalar.activation(rinv2_bc[:], rinv2_bc[:], AF.Sqrt,
                                 bias=eps_col[:], scale=1.0 / HID)
            nc.vector.reciprocal(rinv2_bc[:], rinv2_bc[:])

            # gate/up: 4 groups of 3 m-tiles (g g g | u u u | g g g | u u u)
            act_sb = attn.tile([128, MI // 128, T], bf16, tag="act")
            gate_tiles = {}
            for g in range(NG):
                gu_ps = [psgrp.tile([128, T], f32, tag="grp", name=f"gu_ps{g}_{i}") for i in range(3)]
                for k in range(KT):
                    wtile = wstream.tile([128, 384], bf16, tag="wgu")
                    nc.sync.dma_start(out=wtile[:], in_=p_wgut[g, ts(k, 128), :])
                    for mi in range(3):
                        nc.tensor.matmul(gu_ps[mi][:], wtile[:, ds(mi * 128, 128)],
                                         hw2_sb[:, k],
                                         start=(k == 0), stop=(k == KT - 1))
                is_gate = (g % 2 == 0)
                base = (g // 2) * 3
                for mi in range(3):
                    pre = scratch.tile([128, T], f32, tag="gupre")
                    nc.vector.tensor_tensor(pre[:], gu_ps[mi][:], rinv2_bc[:],
                                            op=ALU.mult)
                    if is_gate:
                        gt = attn.tile([128, T], f32, tag=f"gate{mi}")
                        nc.scalar.activation(gt[:], pre[:], AF.Silu)
                        gate_tiles[base + mi] = gt
                    else:
                        nc.vector.tensor_tensor(act_sb[:, base + mi],
                                                gate_tiles[base + mi][:], pre[:],
                                                op=ALU.mult)

            # down-proj: 8 groups of 2 m-tiles + residual/8 -> cc2_in
            # (RS chunk emitted after each half's groups)
            for g in range(DG):
                d_ps = [psgrp.tile([128, T], f32, tag="grp", name=f"d_ps{g}_{i}") for i in range(2)]
                for k in range(MI // 128):
                    wtile = wstream.tile([128, 256], bf16, tag="wd")
                    nc.sync.dma_start(out=wtile[:], in_=p_wdt[g, ts(k, 128), :])
                    for mi in range(2):
                        nc.tensor.matmul(d_ps[mi][:], wtile[:, ds(mi * 128, 128)],
                                         act_sb[:, k],
                                         start=(k == 0), stop=(k == MI // 128 - 1))
                for mi in range(2):
                    m = g * 2 + mi
                    fin = scratch.tile([128, T], bf16, tag="fin")
                    nc.vector.scalar_tensor_tensor(
                        fin[:], hm_sb[:, m], 1.0 / NC, d_ps[mi][:],
                        op0=ALU.mult, op1=ALU.add)
                    nc.gpsimd.dma_start(out=cc2_ins[m // 4][ts(m % 4, 128), :], in_=fin[:])
                if g % 2 == 1:
                    q = g // 2
                    nc.gpsimd.collective_compute(
                        "ReduceScatter", ALU.add,
                        ins=[cc2_ins[q][:]],
                        outs=[cc2_out[ts(q, 64), :]],
                        replica_groups=groups)

            nc.sync.dma_start(out=p_outh[:], in_=cc2_out[:])

    nc.compile()
    return nc


def _get_nc():
    if "nc" not in _CACHE:
        _patch_walrus_flags()
        _CACHE["nc"] = _build()
    return _CACHE["nc"]


def _shard_inputs(hidden_conv, cos, sin, attn_mask, wq, wk, wv, wo,
                  ln1_w, ln2_w, qn_w, kn_w, w_gate_up, w_down):
    import ml_dtypes
    f = np.float32
    bf = ml_dtypes.bfloat16
    x_fm = np.ascontiguousarray(np.asarray(hidden_conv, f)[0, :, 0, :])   # (2048, 512)
    cos2 = np.asarray(cos, f)[0]
    sin2 = np.asarray(sin, f)[0]
    mask = np.ascontiguousarray(np.asarray(attn_mask, f)[0, 0, :, :T])    # (512, 512)
    qn = np.asarray(qn_w, f); kn = np.asarray(kn_w, f)
    ln1 = np.ascontiguousarray(np.asarray(ln1_w, f).reshape(HID, 1))
    ln2 = np.ascontiguousarray(np.asarray(ln2_w, f).reshape(HID, 1))
    wq = np.asarray(wq, f); wk = np.asarray(wk, f); wv = np.asarray(wv, f)
    wo = np.asarray(wo, f); wgu = np.asarray(w_gate_up, f); wd = np.asarray(w_down, f)

    def rope_tables(w):
        cosw = cos2 * w[None, :]
        sinw = np.concatenate(
            [-sin2[:, 0:64] * w[None, 64:128], sin2[:, 64:128] * w[None, 0:64]], axis=1)
        return np.ascontiguousarray(cosw), np.ascontiguousarray(sinw)

    cosq, sinq = rope_tables(qn)
    cosk, sink = rope_tables(kn)

    in_maps = []
    for c in range(NC):
        wqkvt = np.ascontiguousarray(
            np.concatenate([wq[c * 256:(c + 1) * 256], wk[c * 128:(c + 1) * 128],
                            wv[c * 128:(c + 1) * 128]], 0).T.astype(bf))
        wot = np.ascontiguousarray(wo[:, c * 256:(c + 1) * 256].T.astype(bf))
        # (2048, 1536) -> NG groups of 384 cols: [g0 g1 g2 | u0 u1 u2 | g3.. ]
        wg = wgu[c * MI:(c + 1) * MI].T            # (2048, 768) gate
        wu = wgu[INT + c * MI:INT + (c + 1) * MI].T
        wgut = np.stack([
            wg[:, 0:384], wu[:, 0:384], wg[:, 384:768], wu[:, 384:768]], 0)
        wgut = np.ascontiguousarray(wgut.astype(bf))           # (4, 2048, 384)
        wdt = wd[:, c * MI:(c + 1) * MI].T         # (768, 2048)
        wdt = np.ascontiguousarray(
            wdt.reshape(MI, DG, 256).transpose(1, 0, 2).astype(bf))  # (8, 768, 256)
        in_maps.append(dict(
            x=x_fm, wqkvt=wqkvt, wot=wot, wgut=wgut, wdt=wdt,
            ln1w=ln1, ln2w=ln2, cosq=cosq, sinq=sinq, cosk=cosk, sink=sink,
            mask=mask))
    return in_maps


def kernel(hidden_conv, cos, sin, update_mask, attn_mask, k_cache, v_cache,
           wq, wk, wv, wo, ln1_w, ln2_w, qn_w, kn_w, w_gate_up, w_down):
    import os
    from concourse.bass_utils import run_bass_kernel_spmd

    in_maps = _shard_inputs(hidden_conv, cos, sin, attn_mask, wq, wk, wv, wo,
                            ln1_w, ln2_w, qn_w, kn_w, w_gate_up, w_down)
    nc = _get_nc()
    trace = bool(os.environ.get("KERNEL_TRACE"))
    res = run_bass_kernel_spmd(nc, in_maps, core_ids=list(range(NC)), trace=trace)
    _CACHE["last_res"] = res
    results = res.results

    hid = np.empty((HID, T), np.float32)
    for c in range(NC):
        oh = np.asarray(results[c]["out_hid"], np.float32)
        for q in range(4):
            hid[q * 512 + c * 64:q * 512 + (c + 1) * 64] = oh[q * 64:(q + 1) * 64]
    hid = np.ascontiguousarray(hid.reshape(1, HID, 1, T))
    kf = np.zeros((1, HKV, S, D), np.float32)
    vf = np.zeros((1, HKV, S, D), np.float32)
    for c in range(NC):
        kf[0, c, :T] = results[c]["out_k"]
        vf[0, c, :T] = results[c]["out_v"]
    return hid, kf, vf


# revision 32
# speedup vs baseline: 1.0768x; 1.0452x over previous
"""Trainium2 Bass kernel for nn_ANEPrefillLayer (dense transformer prefill layer).

Tensor-parallel over 8 NeuronCores:
  - q heads: 2 per core; kv head: 1 per core
  - o-proj column-sharded -> AllReduce of attention-output partials
  - MLP intermediate sharded (768 per core) -> ReduceScatter of (mlp + hm/8),
    each core emitting a 256-row feature shard of the final hidden state.
Matmul operands are cast to bf16 on chip (weights cast on host), fp32
accumulation in PSUM.  Activations kept feature-major (features on
partitions); q/k/v projections are produced token-major directly by using the
activation tiles as the stationary operand.
"""
import numpy as np

HID, H, HKV, D, INT, S, T = 2048, 16, 8, 128, 6144, 4096, 512
EPS = 1e-6
SCALE = 1.0 / D**0.5
NC = 8
KT = HID // 128          # 16 k-tiles over hidden
TT = T // 128            # 4 token tiles
QH = H // NC             # 2 q heads per core
MI = INT // NC           # 768 intermediate per core
NG = 4                   # gate/up m-groups (3 m-tiles each)
DG = 8                   # down-proj m-groups (2 m-tiles each)

_CACHE = {}


def _patch_walrus_flags():
    # pair LDWEIGHTS with MATMULs (the default disables the optimization)
    return


def _build():
    import concourse.bass as bass
    import concourse.bass_isa as bass_isa
    import concourse.mybir as mybir
    import concourse.tile as tile
    from concourse.masks import make_identity
    from concourse.bass import ds, ts

    f32 = mybir.dt.float32
    bf16 = mybir.dt.bfloat16
    AF = mybir.ActivationFunctionType
    ALU = mybir.AluOpType

    from concourse import bacc
    nc = bacc.Bacc()

    # ---- DRAM parameters (per-core shards; same graph on all cores) ----
    p_x = nc.declare_dram_parameter("x", [HID, T], f32, isOutput=False)
    p_wqkvt = nc.declare_dram_parameter("wqkvt", [HID, 512], bf16, isOutput=False)
    p_wot = nc.declare_dram_parameter("wot", [256, HID], bf16, isOutput=False)
    # gate/up: column groups [g0 g1 g2 | u0 u1 u2 | g3 g4 g5 | u3 u4 u5]
    p_wgut = nc.declare_dram_parameter("wgut", [NG, HID, 384], bf16, isOutput=False)
    p_wdt = nc.declare_dram_parameter("wdt", [DG, MI, 256], bf16, isOutput=False)
    p_ln1 = nc.declare_dram_parameter("ln1w", [HID, 1], f32, isOutput=False)
    p_ln2 = nc.declare_dram_parameter("ln2w", [HID, 1], f32, isOutput=False)
    p_cosq = nc.declare_dram_parameter("cosq", [T, D], f32, isOutput=False)
    p_sinq = nc.declare_dram_parameter("sinq", [T, D], f32, isOutput=False)
    p_cosk = nc.declare_dram_parameter("cosk", [T, D], f32, isOutput=False)
    p_sink = nc.declare_dram_parameter("sink", [T, D], f32, isOutput=False)
    p_mask = nc.declare_dram_parameter("mask", [T, T], f32, isOutput=False)

    p_outh = nc.declare_dram_parameter("out_hid", [HID // NC, T], bf16, isOutput=True)
    p_outk = nc.declare_dram_parameter("out_k", [T, D], f32, isOutput=True)
    p_outv = nc.declare_dram_parameter("out_v", [T, D], f32, isOutput=True)

    groups = [list(range(NC))]

    with tile.TileContext(nc) as tc:
        with (
            tc.tile_pool(name="dram", bufs=1, space="DRAM") as dram,
            tc.tile_pool(name="const", bufs=1) as const,
            tc.tile_pool(name="xpool", bufs=1) as xpool,
            tc.tile_pool(name="wstream", bufs=8) as wstream,
            tc.tile_pool(name="scratch", bufs=2) as scratch,
            tc.tile_pool(name="small", bufs=8) as small,
            tc.tile_pool(name="attn", bufs=1) as attn,
            tc.tile_pool(name="psacc", bufs=2, space="PSUM") as psacc,
            tc.tile_pool(name="psgrp", bufs=3, space="PSUM") as psgrp,
            tc.tile_pool(name="pstp", bufs=2, space="PSUM") as pstp,
        ):
            # collective bounce buffers (tracked DRAM tiles)
            cc1_sizes = [1024, 1024]
            cc1_ins = [dram.tile([cc1_sizes[i], T], bf16, name=f"cc1_in{i}")
                       for i in range(2)]
            cc1_outs = [dram.tile([cc1_sizes[i], T], bf16, addr_space="Shared",
                                  name=f"cc1_out{i}") for i in range(2)]
            cc2_sizes = [1024, 512, 512]
            cc2_ins = [dram.tile([cc2_sizes[i], T], bf16, name=f"cc2_in{i}")
                       for i in range(3)]
            cc2_out = dram.tile([HID // NC, T], bf16)
            rinv_dram = dram.tile([T, 1], f32)

            x_prime = xpool.tile([128, KT, T], f32, tag="xres", name="x_sb")
            nc.sync.dma_start(out=x_prime[:, 0], in_=p_x[ts(0, 128), :])
            wqkv_sb = const.tile([128, KT, 512], bf16)
            nc.sync.dma_start(out=wqkv_sb[:, 0], in_=p_wqkvt[ts(0, 128), :])

            ident = const.tile([128, 128], bf16)
            make_identity(nc, ident[:])
            eps_col = const.tile([128, 1], f32)
            nc.vector.memset(eps_col[:], EPS)

            # per-feature norm weights: (2048,1) -> (128, 16) column tiles
            ln1_sb = const.tile([128, KT], f32)
            nc.sync.dma_start(out=ln1_sb[:], in_=p_ln1[:].rearrange("(k p) o -> p (k o)", p=128))
            ln2_sb = const.tile([128, KT], f32)
            nc.sync.dma_start(out=ln2_sb[:], in_=p_ln2[:].rearrange("(k p) o -> p (k o)", p=128))

            # ---- phase A: stream x, LN1 stats, xw = x*ln1_w (bf16) ----
            xw_sb = xpool.tile([128, KT, T], bf16, tag="xw")
            x_sb = x_prime
            ss_acc = attn.tile([128, T], f32, tag="ssacc")
            dma_engs = [nc.sync, nc.scalar, nc.gpsimd]
            for k in range(KT):
                if k > 0:
                    dma_engs[k % 3].dma_start(out=x_sb[:, k], in_=p_x[ts(k, 128), :])
                x_t = x_sb[:, k]
                xsq = scratch.tile([128, T], bf16, tag="xsq")
                nc.scalar.activation(xsq[:], x_t, AF.Square)
                if k == 0:
                    nc.vector.tensor_copy(ss_acc[:], xsq[:])
                else:
                    nc.vector.tensor_tensor(ss_acc[:], ss_acc[:], xsq[:], op=ALU.add)
                nc.vector.tensor_tensor(xw_sb[:, k], x_t, ln1_sb[:, ds(k, 1)].to_broadcast((128, T)), op=ALU.mult)

            # rinv broadcast (128, T): 1/sqrt(sum/HID + eps) on every partition
            rinv_bc = attn.tile([128, T], f32, tag="rbc")
            nc.gpsimd.partition_all_reduce(rinv_bc[:], ss_acc[:], channels=128,
                                           reduce_op=bass_isa.ReduceOp.add)
            nc.scalar.activation(rinv_bc[:], rinv_bc[:], AF.Sqrt,
                                 bias=eps_col[:], scale=1.0 / HID)
            nc.vector.reciprocal(rinv_bc[:], rinv_bc[:])
            # token-major (128, TT) copy via DRAM bounce (for the v output)
            nc.sync.dma_start(out=rinv_dram[:], in_=rinv_bc[0:1, :])
            rinv_tm = small.tile([128, TT], f32, tag="rtm")
            nc.sync.dma_start(out=rinv_tm[:],
                              in_=rinv_dram[:].rearrange("(t p) o -> p (t o)", p=128))

            # ---- phase B: QKV (token-major) + head RMS + RoPE ----
            for k in range(1, KT):
                dma_engs[(k + 1) % 3].dma_start(out=wqkv_sb[:, k], in_=p_wqkvt[ts(k, 128), :])

            # RoPE tables + attention mask, token-tiled
            cosq_sb = const.tile([128, TT, D], f32)
            sinq_sb = const.tile([128, TT, D], f32)
            cosk_sb = const.tile([128, TT, D], f32)
            sink_sb = const.tile([128, TT, D], f32)
            mask_sb = const.tile([128, TT, T], f32)
            for t in range(TT):
                nc.sync.dma_start(out=cosq_sb[:, t], in_=p_cosq[ts(t, 128), :])
                nc.sync.dma_start(out=sinq_sb[:, t], in_=p_sinq[ts(t, 128), :])
                nc.sync.dma_start(out=cosk_sb[:, t], in_=p_cosk[ts(t, 128), :])
                nc.sync.dma_start(out=sink_sb[:, t], in_=p_sink[ts(t, 128), :])
                nc.sync.dma_start(out=mask_sb[:, t], in_=p_mask[ts(t, 128), :])


            q_fm = attn.tile([128, QH, T], bf16)   # q feature-major per head
            rq_all = {}
            k_fm = attn.tile([128, T], bf16)
            v_tm = attn.tile([128, TT, D], bf16)   # v token-major (unscaled)

            def rms_rope(dst_bf, src_ap, cos_t, sin_t, tag, dst_f32=None,
                         defer_scale=False):
                # src_ap: (128 tokens, 128 dims) PSUM f32
                sq = scratch.tile([128, D], bf16, tag="rsq")
                ssq = small.tile([128, 1], f32, tag=tag + "ss")
                nc.scalar.activation(sq[:], src_ap, AF.Square, accum_out=ssq[:])
                rr = small.tile([128, 1], f32, tag=tag + "rr", name=f"rr_{tag}")
                nc.scalar.activation(rr[:], ssq[:], AF.Sqrt, bias=eps_col[:], scale=1.0 / D)
                nc.vector.reciprocal(rr[:], rr[:])
                t1 = scratch.tile([128, D], f32, tag="rt1")
                t2 = scratch.tile([128, D], f32, tag="rt2")
                if defer_scale:
                    # rope WITHOUT the rms scale (applied later via exp scale);
                    # runs concurrently with the ssq/sqrt/recip chain
                    nc.vector.tensor_tensor(t1[:], src_ap, cos_t, op=ALU.mult)
                    nc.vector.tensor_tensor(t2[:, 0:64], src_ap[:, 64:128],
                                            sin_t[:, 0:64], op=ALU.mult)
                    nc.vector.tensor_tensor(t2[:, 64:128], src_ap[:, 0:64],
                                            sin_t[:, 64:128], op=ALU.mult)
                else:
                    nc.vector.scalar_tensor_tensor(
                        t1[:], src_ap, rr[:], cos_t, op0=ALU.mult, op1=ALU.mult)
                    nc.vector.scalar_tensor_tensor(
                        t2[:, 0:64], src_ap[:, 64:128], rr[:], sin_t[:, 0:64],
                        op0=ALU.mult, op1=ALU.mult)
                    nc.vector.scalar_tensor_tensor(
                        t2[:, 64:128], src_ap[:, 0:64], rr[:], sin_t[:, 64:128],
                        op0=ALU.mult, op1=ALU.mult)
                if dst_f32 is not None:
                    nc.vector.tensor_tensor(dst_f32, t1[:], t2[:], op=ALU.add)
                    nc.vector.tensor_copy(dst_bf, dst_f32)
                else:
                    nc.vector.tensor_tensor(dst_bf, t1[:], t2[:], op=ALU.add)
                return rr

            for t in range(TT):
                qkv_ps = psgrp.tile([128, 512], f32, tag="grp")
                for k in range(KT):
                    nc.tensor.matmul(qkv_ps[:], xw_sb[:, k, ts(t, 128)], wqkv_sb[:, k],
                                     start=(k == 0), stop=(k == KT - 1))
                q_ps = qkv_ps
                kv_ps = qkv_ps[:, 256:512]

                # q heads: RoPE (rms scale deferred to the exp) -> feature-major
                for h in range(QH):
                    qr = scratch.tile([128, D], bf16, tag="qr")
                    rq = rms_rope(qr[:], qkv_ps[:, ds(h * 128, 128)],
                                  cosq_sb[:, t], sinq_sb[:, t], f"q{h}",
                                  defer_scale=True)
                    rq_all[(t, h)] = rq
                    qt_ps = pstp.tile([128, 128], bf16, tag="tps")
                    nc.tensor.transpose(qt_ps[:], qr[:], ident[:])
                    nc.vector.tensor_copy(q_fm[:, h, ts(t, 128)], qt_ps[:])

                # k head: RMS + RoPE -> out_k (f32) + feature-major (bf16)
                krb = scratch.tile([128, D], bf16, tag="krb")
                krf = scratch.tile([128, D], f32, tag="krf")
                rms_rope(krb[:], qkv_ps[:, 256:384], cosk_sb[:, t], sink_sb[:, t],
                         "k", dst_f32=krf[:])
                nc.sync.dma_start(out=p_outk[ts(t, 128), :], in_=krf[:])
                kt_ps = pstp.tile([128, 128], bf16, tag="tps")
                nc.tensor.transpose(kt_ps[:], krb[:], ident[:])
                nc.vector.tensor_copy(k_fm[:, ts(t, 128)], kt_ps[:])

                # v: unscaled bf16 for attention; ln1-scaled f32 for output
                nc.vector.tensor_copy(v_tm[:, t], qkv_ps[:, 384:512])
                vout = scratch.tile([128, D], f32, tag="vout")
                nc.vector.tensor_tensor(vout[:], qkv_ps[:, 384:512],
                                        rinv_tm[:, ds(t, 1)].to_broadcast((128, D)), op=ALU.mult)
                nc.sync.dma_start(out=p_outv[ts(t, 128), :], in_=vout[:])

            # ---- phase D: attention, heads interleaved, causal-trimmed ----
            # (masked scores are exactly exp(-1e4)=0 in f32, so computing only
            #  the visible lower-triangular tile strip is exact)
            o_fm = attn.tile([128, QH, T], bf16)
            pT_sbs = [attn.tile([128, TT, T], bf16, name=f"pT{h}", tag=f"pT{h}")
                      for h in range(QH)]
            for h in range(QH):
                nc.vector.memset(pT_sbs[h][:], 0.0)
            for t in range(TT):
                W = (t + 1) * 128
                for h in range(QH):
                    s_ps = psacc.tile([128, T], f32, tag="acc")
                    nc.tensor.matmul(s_ps[:, 0:W], q_fm[:, h, ts(t, 128)],
                                     k_fm[:, 0:W], start=True, stop=True)
                    s_sb = scratch.tile([128, T], f32, tag="ssb")
                    nc.vector.scalar_tensor_tensor(
                        s_sb[:, 0:W], s_ps[:, 0:W], SCALE, mask_sb[:, t, 0:W],
                        op0=ALU.mult, op1=ALU.add)
                    p_sb = scratch.tile([128, T], f32, tag="psb")
                    rowsum = small.tile([128, 1], f32, tag="rsum")
                    nc.scalar.activation(p_sb[:, 0:W], s_sb[:, 0:W], AF.Exp,
                                         scale=rq_all[(t, h)][:],
                                         accum_out=rowsum[:])
                    nc.vector.reciprocal(rowsum[:], rowsum[:])
                    pn = scratch.tile([128, T], bf16, tag="pn")
                    nc.vector.scalar_tensor_tensor(
                        pn[:, 0:W], p_sb[:, 0:W], rowsum[:], rinv_bc[:, 0:W],
                        op0=ALU.mult, op1=ALU.mult)
                    for st in range(t + 1):
                        pt_ps = pstp.tile([128, 128], bf16, tag="tps")
                        nc.tensor.transpose(pt_ps[:], pn[:, ts(st, 128)], ident[:])
                        nc.vector.tensor_copy(pT_sbs[h][:, st, ts(t, 128)], pt_ps[:])
            for h in range(QH):
                o_ps = psacc.tile([128, T], f32, tag="acc")
                for st in range(TT):
                    nc.tensor.matmul(o_ps[:], v_tm[:, st], pT_sbs[h][:, st],
                                     start=(st == 0), stop=(st == TT - 1))
                nc.vector.tensor_copy(o_fm[:, h], o_ps[:])

            # ---- phase E: o-proj partials -> AllReduce ----
            wo_sb = const.tile([128, 2, HID], bf16)
            for k in range(2):
                nc.sync.dma_start(out=wo_sb[:, k], in_=p_wot[ts(k, 128), :])
            cc1_mranges = [(0, 8), (8, 16)]
            for q, (m0, m1) in enumerate(cc1_mranges):
                for m in range(m0, m1):
                    ao_ps = psacc.tile([128, T], f32, tag="acc")
                    for k in range(2):
                        nc.tensor.matmul(ao_ps[:], wo_sb[:, k, ts(m, 128)], o_fm[:, k],
                                         start=(k == 0), stop=(k == 1))
                    ao_sb = scratch.tile([128, T], bf16, tag="aosb")
                    nc.any.tensor_copy(ao_sb[:], ao_ps[:])
                    nc.sync.dma_start(out=cc1_ins[q][ts(m - m0, 128), :], in_=ao_sb[:])
                nc.gpsimd.collective_compute(
                    "AllReduce", ALU.add,
                    ins=[cc1_ins[q][:]],
                    outs=[cc1_outs[q][:]],
                    replica_groups=groups)

            # ---- phase F: residual + LN2 + MLP + ReduceScatter ----
            hm_sb = xpool.tile([128, KT, T], f32, tag="hm")
            hw2_sb = xpool.tile([128, KT, T], bf16, tag="xw")   # reuse xw slot
            ss2_acc = attn.tile([128, T], f32, tag="ss2acc")
            for k in range(KT):
                x2 = x_sb[:, k]
                ao = scratch.tile([128, T], bf16, tag="aoin")
                nc.gpsimd.dma_start(out=ao[:], in_=cc1_outs[k // 8][ts(k % 8, 128), :])
                nc.vector.tensor_tensor(hm_sb[:, k], x2, ao[:], op=ALU.add)
                xsq2 = scratch.tile([128, T], bf16, tag="xsq")
                nc.scalar.activation(xsq2[:], hm_sb[:, k], AF.Square)
                if k == 0:
                    nc.vector.tensor_copy(ss2_acc[:], xsq2[:])
                else:
                    nc.vector.tensor_tensor(ss2_acc[:], ss2_acc[:], xsq2[:], op=ALU.add)
                nc.vector.tensor_tensor(hw2_sb[:, k], hm_sb[:, k],
                                        ln2_sb[:, ds(k, 1)].to_broadcast((128, T)),
                                        op=ALU.mult)

            rinv2_bc = attn.tile([128, T], f32, tag="r2bc")
            nc.gpsimd.partition_all_reduce(rinv2_bc[:], ss2_acc[:], channels=128,
                                           reduce_op=bass_isa.ReduceOp.add)
            nc.scalar.activation(rinv2_bc[:], rinv2_bc[:], AF.Sqrt,
                                 bias=eps_col[:], scale=1.0 / HID)
            nc.vector.reciprocal(rinv2_bc[:], rinv2_bc[:])

            # gate/up: 4 groups of 3 m-tiles (g g g | u u u | g g g | u u u)
            act_sb = attn.tile([128, MI // 128, T], bf16, tag="act")
            gate_tiles = {}
            for g in range(NG):
                gu_ps = [psgrp.tile([128, T], f32, tag="grp", name=f"gu_ps{g}_{i}") for i in range(3)]
                for k in range(KT):
                    wtile = wstream.tile([128, 384], bf16, tag="wgu")
                    nc.sync.dma_start(out=wtile[:], in_=p_wgut[g, ts(k, 128), :])
                    for mi in range(3):
                        nc.tensor.matmul(gu_ps[mi][:], wtile[:, ds(mi * 128, 128)],
                                         hw2_sb[:, k],
                                         start=(k == 0), stop=(k == KT - 1))
                is_gate = (g % 2 == 0)
                base = (g // 2) * 3
                for mi in range(3):
                    pre = scratch.tile([128, T], f32, tag="gupre")
                    nc.vector.tensor_tensor(pre[:], gu_ps[mi][:], rinv2_bc[:],
                                            op=ALU.mult)
                    if is_gate:
                        gt = attn.tile([128, T], f32, tag=f"gate{mi}")
                        nc.scalar.activation(gt[:], pre[:], AF.Silu)
                        gate_tiles[base + mi] = gt
                    else:
                        nc.vector.tensor_tensor(act_sb[:, base + mi],
                                                gate_tiles[base + mi][:], pre[:],
                                                op=ALU.mult)

            # down-proj: 8 groups of 2 m-tiles + residual/8 -> cc2_in
            # (RS chunk emitted after each half's groups)
            for g in range(DG):
                d_ps = [psgrp.tile([128, T], f32, tag="grp", name=f"d_ps{g}_{i}") for i in range(2)]
                for k in range(MI // 128):
                    wtile = wstream.tile([128, 256], bf16, tag="wd")
                    nc.sync.dma_start(out=wtile[:], in_=p_wdt[g, ts(k, 128), :])
                    for mi in range(2):
                        nc.tensor.matmul(d_ps[mi][:], wtile[:, ds(mi * 128, 128)],
                                         act_sb[:, k],
                                         start=(k == 0), stop=(k == MI // 128 - 1))
                for mi in range(2):
                    m = g * 2 + mi
                    fin = scratch.tile([128, T], bf16, tag="fin")
                    nc.vector.scalar_tensor_tensor(
                        fin[:], hm_sb[:, m], 1.0 / NC, d_ps[mi][:],
                        op0=ALU.mult, op1=ALU.add)
                    mq = 0 if m < 8 else (1 if m < 12 else 2)
                    nc.gpsimd.dma_start(out=cc2_ins[mq][ts(m - (0, 8, 12)[mq], 128), :], in_=fin[:])
                if g in (3, 5, 7):
                    q = (3, 5, 7).index(g)
                    out_off, out_n = ((0, 128), (128, 64), (192, 64))[q]
                    nc.gpsimd.collective_compute(
                        "ReduceScatter", ALU.add,
                        ins=[cc2_ins[q][:]],
                        outs=[cc2_out[out_off:out_off + out_n, :]],
                        replica_groups=groups)

            nc.sync.dma_start(out=p_outh[:], in_=cc2_out[:])

    nc.compile()
    return nc


def _get_nc():
    if "nc" not in _CACHE:
        _patch_walrus_flags()
        _CACHE["nc"] = _build()
    return _CACHE["nc"]


def _shard_inputs(hidden_conv, cos, sin, attn_mask, wq, wk, wv, wo,
                  ln1_w, ln2_w, qn_w, kn_w, w_gate_up, w_down):
    import ml_dtypes
    f = np.float32
    bf = ml_dtypes.bfloat16
    x_fm = np.ascontiguousarray(np.asarray(hidden_conv, f)[0, :, 0, :])   # (2048, 512)
    cos2 = np.asarray(cos, f)[0]
    sin2 = np.asarray(sin, f)[0]
    mask = np.ascontiguousarray(np.asarray(attn_mask, f)[0, 0, :, :T])    # (512, 512)
    qn = np.asarray(qn_w, f); kn = np.asarray(kn_w, f)
    ln1 = np.ascontiguousarray(np.asarray(ln1_w, f).reshape(HID, 1))
    ln2 = np.ascontiguousarray(np.asarray(ln2_w, f).reshape(HID, 1))
    wq = np.asarray(wq, f); wk = np.asarray(wk, f); wv = np.asarray(wv, f)
    wo = np.asarray(wo, f); wgu = np.asarray(w_gate_up, f); wd = np.asarray(w_down, f)

    def rope_tables(w):
        cosw = cos2 * w[None, :]
        sinw = np.concatenate(
            [-sin2[:, 0:64] * w[None, 64:128], sin2[:, 64:128] * w[None, 0:64]], axis=1)
        return np.ascontiguousarray(cosw), np.ascontiguousarray(sinw)

    cosq, sinq = rope_tables(qn)
    cosk, sink = rope_tables(kn)

    in_maps = []
    for c in range(NC):
        wqkvt = np.ascontiguousarray(
            np.concatenate([wq[c * 256:(c + 1) * 256], wk[c * 128:(c + 1) * 128],
                            wv[c * 128:(c + 1) * 128]], 0).T.astype(bf))
        wot = np.ascontiguousarray(wo[:, c * 256:(c + 1) * 256].T.astype(bf))
        # (2048, 1536) -> NG groups of 384 cols: [g0 g1 g2 | u0 u1 u2 | g3.. ]
        wg = wgu[c * MI:(c + 1) * MI].T            # (2048, 768) gate
        wu = wgu[INT + c * MI:INT + (c + 1) * MI].T
        wgut = np.stack([
            wg[:, 0:384], wu[:, 0:384], wg[:, 384:768], wu[:, 384:768]], 0)
        wgut = np.ascontiguousarray(wgut.astype(bf))           # (4, 2048, 384)
        wdt = wd[:, c * MI:(c + 1) * MI].T         # (768, 2048)
        wdt = np.ascontiguousarray(
            wdt.reshape(MI, DG, 256).transpose(1, 0, 2).astype(bf))  # (8, 768, 256)
        in_maps.append(dict(
            x=x_fm, wqkvt=wqkvt, wot=wot, wgut=wgut, wdt=wdt,
            ln1w=ln1, ln2w=ln2, cosq=cosq, sinq=sinq, cosk=cosk, sink=sink,
            mask=mask))
    return in_maps


def kernel(hidden_conv, cos, sin, update_mask, attn_mask, k_cache, v_cache,
           wq, wk, wv, wo, ln1_w, ln2_w, qn_w, kn_w, w_gate_up, w_down):
    import os
    from concourse.bass_utils import run_bass_kernel_spmd

    in_maps = _shard_inputs(hidden_conv, cos, sin, attn_mask, wq, wk, wv, wo,
                            ln1_w, ln2_w, qn_w, kn_w, w_gate_up, w_down)
    nc = _get_nc()
    trace = bool(os.environ.get("KERNEL_TRACE"))
    res = run_bass_kernel_spmd(nc, in_maps, core_ids=list(range(NC)), trace=trace)
    _CACHE["last_res"] = res
    results = res.results

    hid = np.empty((HID, T), np.float32)
    for c in range(NC):
        oh = np.asarray(results[c]["out_hid"], np.float32)
        hid[c * 128:(c + 1) * 128] = oh[0:128]
        hid[1024 + c * 64:1024 + (c + 1) * 64] = oh[128:192]
        hid[1536 + c * 64:1536 + (c + 1) * 64] = oh[192:256]
    hid = np.ascontiguousarray(hid.reshape(1, HID, 1, T))
    kf = np.zeros((1, HKV, S, D), np.float32)
    vf = np.zeros((1, HKV, S, D), np.float32)
    for c in range(NC):
        kf[0, c, :T] = results[c]["out_k"]
        vf[0, c, :T] = results[c]["out_v"]
    return hid, kf, vf
